# revision 41
# baseline (speedup 1.0000x reference)
"""Bahdanau-style additive attention kernel for Trainium2 (8 NeuronCores).

ctx, att = attention(query, value, mask, Wk, Wq, bq, Wo, bo)
  k      = value @ Wk                          [B,S,H]
  q      = query @ Wq + bq                     [B,H]
  scores = tanh(q[:,None,:] + k) @ Wo + bo     [B,S]
  att    = softmax(mask ? scores : -1e9)       [B,S]
  ctx    = sum_s att * value                   [B,V]

Sharding: data-parallel over batch (4 batches per core, 8 cores).
Single pass over `value` per core: cast-load fp16, xbar-DMA transpose for the
k-projection (contraction dim on partitions), max-free softmax (scores are
bounded by sum|Wo|+|bo| <= ~11.4, we shift the exponent by -4 so fp16 weights
cannot overflow), and the weighted sum is accumulated from the same value
tiles so HBM traffic is one read of `value`.
"""

import os
import sys

for _r in ("/opt/trn_rl_repo", "/root/.axon_site/_ro/trn_rl_repo"):
    if os.path.isdir(_r):
        for _p in (_r, os.path.join(_r, "concourse")):
            if _p not in sys.path:
                sys.path.insert(0, _p)
        break

import numpy as np

import concourse.bacc as bacc
import concourse.mybir as mybir
from concourse import tile
from concourse.bass_utils import run_bass_kernel_spmd

B, S, QD, VD, HD = 32, 4096, 512, 512, 512
NCORES = 8
BPC = B // NCORES      # batches per core
SBLK = 512             # seq positions per block
NBLK = S // SBLK       # 8 blocks per batch
NT = SBLK // 128       # 4 seq tiles per block
NVC = VD // 128        # value-dim chunks
NHC = HD // 128        # hidden-dim chunks
NQC = QD // 128        # query-dim chunks
EXP_SHIFT = -4.0       # exp(scores + bo + EXP_SHIFT); cancels in att/ctx

F32 = mybir.dt.float32
F16 = mybir.dt.float16
U8 = mybir.dt.uint8
AF = mybir.ActivationFunctionType
ALU = mybir.AluOpType

_CACHE = {}


def _build():
    nc = bacc.Bacc(None, target_bir_lowering=False, debug=False,
                   num_swdge_queues=4)
    value = nc.dram_tensor("value_s", [BPC, S, VD], F32, kind="ExternalInput")
    query = nc.dram_tensor("query_s", [BPC, QD], F32, kind="ExternalInput")
    mask = nc.dram_tensor("mask_s", [BPC, S], U8, kind="ExternalInput")
    wk = nc.dram_tensor("Wk", [VD, HD], F32, kind="ExternalInput")
    wq = nc.dram_tensor("Wq", [QD, HD], F32, kind="ExternalInput")
    bq = nc.dram_tensor("bq", [1, HD], F32, kind="ExternalInput")
    wo = nc.dram_tensor("Wo", [1, HD], F32, kind="ExternalInput")
    bo = nc.dram_tensor("bo", [1, 1], F32, kind="ExternalInput")
    ctx_o = nc.dram_tensor("ctx_s", [BPC, VD], F32, kind="ExternalOutput")
    att_o = nc.dram_tensor("att_s", [BPC, S], F32, kind="ExternalOutput")

    TAIL_LAG = 2
    EPI_LAG = 4

    with tile.TileContext(nc) as tc:
        with tc.tile_pool(name="persist", bufs=1) as pp, \
             tc.tile_pool(name="vn", bufs=5) as vn_pool, \
             tc.tile_pool(name="vt", bufs=6) as vt_pool, \
             tc.tile_pool(name="ht", bufs=4) as ht_pool, \
             tc.tile_pool(name="rows", bufs=2) as rows, \
             tc.tile_pool(name="kt_ps", bufs=3, space="PSUM") as kt_pool, \
             tc.tile_pool(name="sc_ps", bufs=2, space="PSUM") as sc_pool, \
             tc.tile_pool(name="pt_ps", bufs=2, space="PSUM") as pt_pool, \
             tc.tile_pool(name="cx_ps", bufs=1, space="PSUM") as cx_pool:

            # ---------- superblock loader (2 seq blocks per DMA) ----------
            supers = {}

            def ensure_super(b, sbk):
                key = (b, sbk)
                if key in supers:
                    return supers[key]
                s0 = sbk * 2 * SBLK
                v_nat = vn_pool.tile([128, 2 * NT, VD], F16, tag="vn",
                                     name=f"vn_{b}_{sbk}")
                # (p t) order: partition p holds 8 consecutive seq rows, so
                # each partition reads one 16KB-contiguous DRAM run.
                # v_nat[p, t, v] = value[b, s0 + 8p + t, v]
                src = value[b, s0:s0 + 2 * SBLK, :].rearrange(
                    "(p t) v -> p t v", p=128)
                if sbk == 0:
                    # latency-critical first superblock of a batch: split per
                    # half so the first transpose starts sooner
                    for h in range(2):
                        nc.gpsimd.dma_start(
                            out=v_nat[:, 4 * h:4 * h + 4, :],
                            in_=src[:, 4 * h:4 * h + 4, :])
                else:
                    nc.gpsimd.dma_start(out=v_nat[:], in_=src)
                vT4s = []
                for h in range(2):
                    vT4 = vt_pool.tile([128, NT, NVC, 128], F16, tag="vt4",
                                       name=f"vT4_{b}_{sbk}_{h}")
                    # vT4[p,t,c,q] = value[b, s0 + 8q + 4h + t, c*128+p]
                    # (block j = 2*sbk+h covers s = s0 + 8q + 4h + t;
                    #  score column n = t*128 + q)
                    nc.sync.dma_start(
                        out=vT4[:].rearrange("p t c s -> p (t c) s"),
                        in_=v_nat[:, h * NT:(h + 1) * NT, :].rearrange(
                            "p t v -> p (t v)"),
                        transpose=True)
                    vT4s.append(vT4)
                supers[key] = (v_nat, vT4s)
                return supers[key]

            # ---------- persistent constants ----------
            wk16 = pp.tile([128, NVC, HD], F16)
            ones16 = pp.tile([1, BPC], F16)
            onesf = pp.tile([1, 1], F32)
            woT16 = pp.tile([128, NHC], F16)
            bo_sb = pp.tile([1, 1], F32)
            bo4 = pp.tile([1, 1], F32)
            qt_sb = pp.tile([128, NHC, BPC], F32)
            wq16 = pp.tile([128, NQC, HD], F16)
            qT16 = pp.tile([128, NQC, BPC], F16)
            bq16 = pp.tile([1, HD], F16)
            wo16 = pp.tile([1, HD], F16)

            # prefetch the first superblock before any constant loads
            ensure_super(0, 0)

            # ---------- preamble ----------
            for vc in range(NVC):
                nc.gpsimd.dma_start(out=wk16[:, vc, :],
                                    in_=wk[vc * 128:(vc + 1) * 128, :])
            nc.vector.memset(ones16[:], 1.0)
            nc.vector.memset(onesf[:], 1.0)
            nc.gpsimd.dma_start(out=bo_sb[:], in_=bo[:, :])
            nc.vector.tensor_scalar_add(bo4[:], bo_sb[:], EXP_SHIFT)
            nc.gpsimd.dma_start(out=wq16[:],
                                in_=wq[:, :].rearrange("(c p) h -> p c h", p=128))
            for qc in range(NQC):
                nc.gpsimd.dma_start(
                    out=qT16[:, qc, :],
                    in_=query[:, qc * 128:(qc + 1) * 128].rearrange("b p -> p b"))
            nc.gpsimd.dma_start(out=bq16[:], in_=bq[:, :])
            nc.gpsimd.dma_start(out=wo16[:], in_=wo[:, :])
            woT_ps = pt_pool.tile([128, NHC], F32, tag="ptps")
            for hc in range(NHC):
                nc.tensor.matmul(woT_ps[:, hc:hc + 1],
                                 wo16[:, hc * 128:(hc + 1) * 128], ones16[:, :1])
            nc.vector.tensor_copy(woT16[:], woT_ps[:])
            for hc in range(NHC):
                qt_ps = pt_pool.tile([128, BPC], F32, tag="ptps",
                                     name=f"qt_ps_{hc}")
                for qc in range(NQC):
                    nc.tensor.matmul(qt_ps[:],
                                     wq16[:, qc, hc * 128:(hc + 1) * 128],
                                     qT16[:, qc, :],
                                     start=(qc == 0), stop=False)
                nc.tensor.matmul(qt_ps[:], bq16[:, hc * 128:(hc + 1) * 128],
                                 ones16[:, :BPC], start=False, stop=True)
                nc.vector.tensor_copy(qt_sb[:, hc, :], qt_ps[:])

            # ---------- software-pipelined main loop ----------
            batch_state = {}

            def get_batch(b):
                if b not in batch_state:
                    mask_row = rows.tile([1, S], F16, tag="mask",
                                         name=f"mask_{b}")
                    nc.gpsimd.dma_start(out=mask_row[:], in_=mask[b:b + 1, :])
                    batch_state[b] = dict(
                        mask_row=mask_row,
                        p_row=rows.tile([1, S], F32, tag="p", name=f"p_{b}"),
                        l_parts=rows.tile([1, NBLK], F32, tag="lp",
                                          name=f"lp_{b}"),
                        ctx_acc=rows.tile([1, VD], F32, tag="ctxacc",
                                          name=f"cacc_{b}"),
                    )
                return batch_state[b]

            def emit_head(b, j):
                get_batch(b)
                v_nat, vT4s = ensure_super(b, j // 2)
                h = j % 2
                hT = ht_pool.tile([128, NHC, SBLK], F16, tag="ht",
                                  name=f"hT_{b}_{j}")
                for hc in range(NHC):
                    kt_ps = kt_pool.tile([128, SBLK], F32, tag="kt",
                                         name=f"kt_{b}_{j}_{hc}")
                    for vc in range(NVC):
                        nc.tensor.matmul(kt_ps[:],
                                         wk16[:, vc, hc * 128:(hc + 1) * 128],
                                         vT4s[h][:, :, vc, :],
                                         start=(vc == 0), stop=(vc == NVC - 1))
                    nc.scalar.activation(hT[:, hc, :], kt_ps[:], AF.Tanh,
                                         bias=qt_sb[:, hc, b:b + 1])
                return dict(b=b, j=j, h=h, hT=hT, v_nat=v_nat)

            def emit_tail(st):
                b, j, h, hT, v_nat = st["b"], st["j"], st["h"], st["hT"], st["v_nat"]
                bs = get_batch(b)
                sj = j * SBLK
                sc_ps = sc_pool.tile([1, SBLK], F32, tag="sc",
                                     name=f"sc_{b}_{j}")
                for hc in range(NHC):
                    nc.tensor.matmul(sc_ps[:], woT16[:, hc:hc + 1], hT[:, hc, :],
                                     start=(hc == 0), stop=(hc == NHC - 1))
                e_blk = rows.tile([1, SBLK], F32, tag="eblk", name=f"e_{b}_{j}")
                nc.scalar.activation(e_blk[:], sc_ps[:], AF.Exp, bias=bo4[:, :1])
                # mask in permuted order: column n = t*128+q <-> s = s0+8q+4h+t
                s0 = (j // 2) * 2 * SBLK
                mview = bs["mask_row"][0:1, s0:s0 + 2 * SBLK].rearrange(
                    "o (q x) -> o q x", q=128)[:, :, 4 * h:4 * h + 4].rearrange(
                    "o q t -> o t q")
                nc.vector.scalar_tensor_tensor(
                    bs["p_row"][:, sj:sj + SBLK], e_blk[:], 1.0,
                    mview,
                    op0=ALU.mult, op1=ALU.mult,
                    accum_out=bs["l_parts"][:, j:j + 1])
                pt_ps = pt_pool.tile([128, NT], F32, tag="ptps",
                                     name=f"pt_{b}_{j}")
                for t in range(NT):
                    nc.tensor.matmul(pt_ps[:, t:t + 1],
                                     bs["p_row"][:, sj + t * 128:sj + (t + 1) * 128],
                                     onesf[:, :1])
                pT16 = rows.tile([128, NT], F16, tag="pt16", name=f"pt16_{b}_{j}")
                nc.vector.tensor_copy(pT16[:], pt_ps[:])
                ctx_ps = cx_pool.tile([1, VD], F32, tag="cx", name=f"cx_{b}_{j}")
                for t in range(NT):
                    nc.tensor.matmul(ctx_ps[:], pT16[:, t:t + 1],
                                     v_nat[:, h * NT + t, :],
                                     start=(t == 0), stop=(t == NT - 1))
                if j == 0:
                    nc.vector.tensor_copy(bs["ctx_acc"][:], ctx_ps[:])
                else:
                    nc.vector.tensor_tensor(bs["ctx_acc"][:], bs["ctx_acc"][:],
                                            ctx_ps[:], op=ALU.add)
            def emit_epilogue(b):
                bs = batch_state[b]
                l_sb = rows.tile([1, 1], F32, tag="l", name=f"l_{b}")
                nc.vector.reduce_sum(l_sb[:], bs["l_parts"][:],
                                     axis=mybir.AxisListType.X)
                rinv = rows.tile([1, 1], F32, tag="rinv", name=f"rinv_{b}")
                nc.vector.reciprocal(rinv[:], l_sb[:])
                # unpermute p_row (sb hh t q order) to natural s order
                att_row = rows.tile([1, S], F32, tag="att", name=f"att_{b}")
                pview = bs["p_row"][0:1, :].rearrange(
                    "o (sb hh t q) -> o sb q hh t",
                    sb=NBLK // 2, hh=2, t=NT, q=128)
                nc.vector.tensor_scalar_mul(att_row[:], pview, rinv[:, :1])
                nc.gpsimd.dma_start(out=att_o[b:b + 1, :], in_=att_row[:])
                ctx_sb = rows.tile([1, VD], F32, tag="ctx", name=f"ctxsb_{b}")
                nc.vector.tensor_scalar_mul(ctx_sb[:], bs["ctx_acc"][:],
                                            rinv[:, :1])
                nc.gpsimd.dma_start(out=ctx_o[b:b + 1, :], in_=ctx_sb[:])

            blocks = [(b, j) for b in range(BPC) for j in range(NBLK)]
            pending = []
            epi_queue = []  # (batch, tails_remaining_until_emit)
            done_tails = 0

            def tail_done(st):
                nonlocal done_tails
                emit_tail(st)
                done_tails += 1
                if st["j"] == NBLK - 1:
                    epi_queue.append((st["b"], done_tails + EPI_LAG))
                while epi_queue and epi_queue[0][1] <= done_tails:
                    emit_epilogue(epi_queue.pop(0)[0])

            for (b, j) in blocks:
                pending.append(emit_head(b, j))
                if len(pending) > TAIL_LAG:
                    tail_done(pending.pop(0))
            for st in pending:
                tail_done(st)
            while epi_queue:
                emit_epilogue(epi_queue.pop(0)[0])

    nc.compile()
    return nc


def _get_nc():
    if "nc" not in _CACHE:
        _CACHE["nc"] = _build()
    return _CACHE["nc"]


def run(inputs, trace=False, tmpdir=None):
    nc = _get_nc()
    value = np.ascontiguousarray(np.asarray(inputs["value"], dtype=np.float32))
    query = np.ascontiguousarray(np.asarray(inputs["query"], dtype=np.float32))
    mask = np.ascontiguousarray(np.asarray(inputs["mask"]).astype(np.uint8))
    wk = np.ascontiguousarray(np.asarray(inputs["Wk"], dtype=np.float32))
    wq = np.ascontiguousarray(np.asarray(inputs["Wq"], dtype=np.float32))
    bq = np.asarray(inputs["bq"], dtype=np.float32).reshape(1, HD)
    wo = np.asarray(inputs["Wo"], dtype=np.float32).reshape(1, HD)
    bo = np.asarray(inputs["bo"], dtype=np.float32).reshape(1, 1)

    in_maps = []
    for c in range(NCORES):
        sl = slice(c * BPC, (c + 1) * BPC)
        in_maps.append({
            "value_s": value[sl],
            "query_s": query[sl],
            "mask_s": mask[sl],
            "Wk": wk, "Wq": wq, "bq": bq, "Wo": wo, "bo": bo,
        })
    res = run_bass_kernel_spmd(nc, in_maps, core_ids=list(range(NCORES)),
                               trace=trace, tmpdir=tmpdir)
    ctx = np.concatenate([r["ctx_s"] for r in res.results], axis=0)
    att = np.concatenate([r["att_s"] for r in res.results], axis=0)
    return (ctx, att), res


def kernel(**inputs):
    (ctx, att), _ = run(inputs)
    return ctx, att


if __name__ == "__main__":
    rng = np.random.default_rng(0)
    demo = {
        "query": rng.standard_normal((B, QD)).astype(np.float32),
        "value": rng.standard_normal((B, S, VD)).astype(np.float32),
        "mask": np.ones((B, S), dtype=bool),
        "Wk": rng.uniform(-1, 1, (VD, HD)).astype(np.float32) / np.sqrt(VD),
        "Wq": rng.uniform(-1, 1, (QD, HD)).astype(np.float32) / np.sqrt(QD),
        "bq": rng.uniform(-1, 1, HD).astype(np.float32) / np.sqrt(QD),
        "Wo": rng.uniform(-1, 1, HD).astype(np.float32) / np.sqrt(HD),
        "bo": np.float32(0.01),
    }
    ctx, att = kernel(**demo)
    print("ctx", ctx.shape, "att", att.shape)


# revision 43
# speedup vs baseline: 27729.3112x; 27729.3112x over previous
"""Bahdanau-style additive attention kernel for Trainium2 (8 NeuronCores).

ctx, att = attention(query, value, mask, Wk, Wq, bq, Wo, bo)
  k      = value @ Wk                          [B,S,H]
  q      = query @ Wq + bq                     [B,H]
  scores = tanh(q[:,None,:] + k) @ Wo + bo     [B,S]
  att    = softmax(mask ? scores : -1e9)       [B,S]
  ctx    = sum_s att * value                   [B,V]

Sharding: data-parallel over batch (4 batches per core, 8 cores).
Single pass over `value` per core: cast-load fp16, xbar-DMA transpose for the
k-projection (contraction dim on partitions), max-free softmax (scores are
bounded by sum|Wo|+|bo| <= ~11.4, we shift the exponent by -4 so fp16 weights
cannot overflow), and the weighted sum is accumulated from the same value
tiles so HBM traffic is one read of `value`.
"""

import os
import sys

for _r in ("/opt/trn_rl_repo", "/root/.axon_site/_ro/trn_rl_repo"):
    if os.path.isdir(_r):
        for _p in (_r, os.path.join(_r, "concourse")):
            if _p not in sys.path:
                sys.path.insert(0, _p)
        break

import numpy as np

import concourse.bacc as bacc
import concourse.mybir as mybir
from concourse import tile
from concourse.bass_utils import run_bass_kernel_spmd

B, S, QD, VD, HD = 32, 4096, 512, 512, 512
NCORES = 8
BPC = B // NCORES      # batches per core
SBLK = 512             # seq positions per block
NBLK = S // SBLK       # 8 blocks per batch
NT = SBLK // 128       # 4 seq tiles per block
NVC = VD // 128        # value-dim chunks
NHC = HD // 128        # hidden-dim chunks
NQC = QD // 128        # query-dim chunks
EXP_SHIFT = -4.0       # exp(scores + bo + EXP_SHIFT); cancels in att/ctx

F32 = mybir.dt.float32
F16 = mybir.dt.float16
U8 = mybir.dt.uint8
AF = mybir.ActivationFunctionType
ALU = mybir.AluOpType

_CACHE = {}


def _build():
    nc = bacc.Bacc(None, target_bir_lowering=False, debug=False,
                   num_swdge_queues=4)
    value = nc.dram_tensor("value_s", [BPC, S, VD], F32, kind="ExternalInput")
    query = nc.dram_tensor("query_s", [BPC, QD], F32, kind="ExternalInput")
    mask = nc.dram_tensor("mask_s", [BPC, S], U8, kind="ExternalInput")
    wk = nc.dram_tensor("Wk", [VD, HD], F32, kind="ExternalInput")
    wq = nc.dram_tensor("Wq", [QD, HD], F32, kind="ExternalInput")
    bq = nc.dram_tensor("bq", [1, HD], F32, kind="ExternalInput")
    wo = nc.dram_tensor("Wo", [1, HD], F32, kind="ExternalInput")
    bo = nc.dram_tensor("bo", [1, 1], F32, kind="ExternalInput")
    ctx_o = nc.dram_tensor("ctx_s", [BPC, VD], F32, kind="ExternalOutput")
    att_o = nc.dram_tensor("att_s", [BPC, S], F32, kind="ExternalOutput")

    TAIL_LAG = 2
    EPI_LAG = 4

    with tile.TileContext(nc) as tc:
        with tc.tile_pool(name="persist", bufs=1) as pp, \
             tc.tile_pool(name="vn", bufs=5) as vn_pool, \
             tc.tile_pool(name="vt", bufs=3) as vt_pool, \
             tc.tile_pool(name="ht", bufs=4) as ht_pool, \
             tc.tile_pool(name="rows", bufs=2) as rows, \
             tc.tile_pool(name="kt_ps", bufs=3, space="PSUM") as kt_pool, \
             tc.tile_pool(name="sc_ps", bufs=2, space="PSUM") as sc_pool, \
             tc.tile_pool(name="pt_ps", bufs=2, space="PSUM") as pt_pool, \
             tc.tile_pool(name="cx_ps", bufs=1, space="PSUM") as cx_pool:

            # ---------- superblock loader (2 seq blocks per DMA) ----------
            supers = {}

            def ensure_super(b, sbk):
                key = (b, sbk)
                if key in supers:
                    return supers[key]
                s0 = sbk * 2 * SBLK
                v_nat = vn_pool.tile([128, 2 * NT, VD], F16, tag="vn",
                                     name=f"vn_{b}_{sbk}")
                # (p t) order: partition p holds 8 consecutive seq rows, so
                # each partition reads one 16KB-contiguous DRAM run.
                # v_nat[p, t, v] = value[b, s0 + 8p + t, v]
                src = value[b, s0:s0 + 2 * SBLK, :].rearrange(
                    "(p t) v -> p t v", p=128)
                if sbk == 0:
                    # latency-critical first superblock of a batch: split per
                    # half so the first transpose starts sooner
                    for h in range(2):
                        nc.gpsimd.dma_start(
                            out=v_nat[:, 4 * h:4 * h + 4, :],
                            in_=src[:, 4 * h:4 * h + 4, :])
                else:
                    nc.gpsimd.dma_start(out=v_nat[:], in_=src)
                # vT8[p,tg,c,q] = value[b, s0 + 8q + tg, c*128+p], tg in [0,8)
                # (block j = 2*sbk+h owns tg = 4h+t; score column n = t*128+q)
                vT8 = vt_pool.tile([128, 2 * NT, NVC, 128], F16, tag="vt4",
                                   name=f"vT8_{b}_{sbk}")
                if sbk == 0:
                    # per-half transposes so the first k-matmuls start sooner
                    for h in range(2):
                        nc.sync.dma_start(
                            out=vT8[:, h * NT:(h + 1) * NT, :, :].rearrange(
                                "p t c s -> p (t c) s"),
                            in_=v_nat[:, h * NT:(h + 1) * NT, :].rearrange(
                                "p t v -> p (t v)"),
                            transpose=True)
                else:
                    nc.sync.dma_start(
                        out=vT8[:].rearrange("p t c s -> p (t c) s"),
                        in_=v_nat[:].rearrange("p t v -> p (t v)"),
                        transpose=True)
                supers[key] = (v_nat, vT8)
                return supers[key]

            # ---------- persistent constants ----------
            wk16 = pp.tile([128, NVC, HD], F16)
            ones16 = pp.tile([1, BPC], F16)
            onesf = pp.tile([1, 1], F32)
            woT16 = pp.tile([128, NHC], F16)
            bo_sb = pp.tile([1, 1], F32)
            bo4 = pp.tile([1, 1], F32)
            qt_sb = pp.tile([128, NHC, BPC], F32)
            wq16 = pp.tile([128, NQC, HD], F16)
            qT16 = pp.tile([128, NQC, BPC], F16)
            bq16 = pp.tile([1, HD], F16)
            wo16 = pp.tile([1, HD], F16)

            # prefetch the first superblock before any constant loads
            ensure_super(0, 0)

            # ---------- preamble ----------
            nc.vector.memset(ones16[:], 1.0)
            nc.vector.memset(onesf[:], 1.0)
            nc.gpsimd.dma_start(out=wo16[:], in_=wo[:, :])
            nc.gpsimd.dma_start(out=bq16[:], in_=bq[:, :])
            nc.gpsimd.dma_start(out=bo_sb[:], in_=bo[:, :])
            nc.vector.tensor_scalar_add(bo4[:], bo_sb[:], EXP_SHIFT)
            for qc in range(NQC):
                nc.gpsimd.dma_start(
                    out=qT16[:, qc, :],
                    in_=query[:, qc * 128:(qc + 1) * 128].rearrange("b p -> p b"))
            nc.gpsimd.dma_start(out=wq16[:],
                                in_=wq[:, :].rearrange("(c p) h -> p c h", p=128))
            for vc in range(NVC):
                nc.gpsimd.dma_start(out=wk16[:, vc, :],
                                    in_=wk[vc * 128:(vc + 1) * 128, :])
            woT_ps = pt_pool.tile([128, NHC], F32, tag="ptps")
            for hc in range(NHC):
                nc.tensor.matmul(woT_ps[:, hc:hc + 1],
                                 wo16[:, hc * 128:(hc + 1) * 128], ones16[:, :1])
            nc.vector.tensor_copy(woT16[:], woT_ps[:])
            for hc in range(NHC):
                qt_ps = pt_pool.tile([128, BPC], F32, tag="ptps",
                                     name=f"qt_ps_{hc}")
                for qc in range(NQC):
                    nc.tensor.matmul(qt_ps[:],
                                     wq16[:, qc, hc * 128:(hc + 1) * 128],
                                     qT16[:, qc, :],
                                     start=(qc == 0), stop=False)
                nc.tensor.matmul(qt_ps[:], bq16[:, hc * 128:(hc + 1) * 128],
                                 ones16[:, :BPC], start=False, stop=True)
                nc.vector.tensor_copy(qt_sb[:, hc, :], qt_ps[:])

            # ---------- software-pipelined main loop ----------
            batch_state = {}

            def get_batch(b):
                if b not in batch_state:
                    mask_row = rows.tile([1, S], F16, tag="mask",
                                         name=f"mask_{b}")
                    nc.gpsimd.dma_start(out=mask_row[:], in_=mask[b:b + 1, :])
                    batch_state[b] = dict(
                        mask_row=mask_row,
                        p_row=rows.tile([1, S], F32, tag="p", name=f"p_{b}"),
                        l_parts=rows.tile([1, NBLK], F32, tag="lp",
                                          name=f"lp_{b}"),
                        ctx_acc=rows.tile([1, VD], F32, tag="ctxacc",
                                          name=f"cacc_{b}"),
                    )
                return batch_state[b]

            def emit_head(b, j):
                get_batch(b)
                v_nat, vT8 = ensure_super(b, j // 2)
                h = j % 2
                hT = ht_pool.tile([128, NHC, SBLK], F16, tag="ht",
                                  name=f"hT_{b}_{j}")
                for hc in range(NHC):
                    kt_ps = kt_pool.tile([128, SBLK], F32, tag="kt",
                                         name=f"kt_{b}_{j}_{hc}")
                    for vc in range(NVC):
                        nc.tensor.matmul(kt_ps[:],
                                         wk16[:, vc, hc * 128:(hc + 1) * 128],
                                         vT8[:, h * NT:(h + 1) * NT, vc, :],
                                         start=(vc == 0), stop=(vc == NVC - 1))
                    nc.scalar.activation(hT[:, hc, :], kt_ps[:], AF.Tanh,
                                         bias=qt_sb[:, hc, b:b + 1])
                return dict(b=b, j=j, h=h, hT=hT, v_nat=v_nat)

            def emit_tail(st):
                b, j, h, hT, v_nat = st["b"], st["j"], st["h"], st["hT"], st["v_nat"]
                bs = get_batch(b)
                sj = j * SBLK
                sc_ps = sc_pool.tile([1, SBLK], F32, tag="sc",
                                     name=f"sc_{b}_{j}")
                for hc in range(NHC):
                    nc.tensor.matmul(sc_ps[:], woT16[:, hc:hc + 1], hT[:, hc, :],
                                     start=(hc == 0), stop=(hc == NHC - 1))
                e_blk = rows.tile([1, SBLK], F32, tag="eblk", name=f"e_{b}_{j}")
                nc.scalar.activation(e_blk[:], sc_ps[:], AF.Exp, bias=bo4[:, :1])
                # mask in permuted order: column n = t*128+q <-> s = s0+8q+4h+t
                s0 = (j // 2) * 2 * SBLK
                mview = bs["mask_row"][0:1, s0:s0 + 2 * SBLK].rearrange(
                    "o (q x) -> o q x", q=128)[:, :, 4 * h:4 * h + 4].rearrange(
                    "o q t -> o t q")
                nc.vector.scalar_tensor_tensor(
                    bs["p_row"][:, sj:sj + SBLK], e_blk[:], 1.0,
                    mview,
                    op0=ALU.mult, op1=ALU.mult,
                    accum_out=bs["l_parts"][:, j:j + 1])
                pt_ps = pt_pool.tile([128, NT], F32, tag="ptps",
                                     name=f"pt_{b}_{j}")
                for t in range(NT):
                    nc.tensor.matmul(pt_ps[:, t:t + 1],
                                     bs["p_row"][:, sj + t * 128:sj + (t + 1) * 128],
                                     onesf[:, :1])
                pT16 = rows.tile([128, NT], F16, tag="pt16", name=f"pt16_{b}_{j}")
                nc.vector.tensor_copy(pT16[:], pt_ps[:])
                ctx_ps = cx_pool.tile([1, VD], F32, tag="cx", name=f"cx_{b}_{j}")
                for t in range(NT):
                    nc.tensor.matmul(ctx_ps[:], pT16[:, t:t + 1],
                                     v_nat[:, h * NT + t, :],
                                     start=(t == 0), stop=(t == NT - 1))
                if j == 0:
                    nc.vector.tensor_copy(bs["ctx_acc"][:], ctx_ps[:])
                else:
                    nc.vector.tensor_tensor(bs["ctx_acc"][:], bs["ctx_acc"][:],
                                            ctx_ps[:], op=ALU.add)
            def emit_epilogue(b):
                bs = batch_state[b]
                l_sb = rows.tile([1, 1], F32, tag="l", name=f"l_{b}")
                nc.vector.reduce_sum(l_sb[:], bs["l_parts"][:],
                                     axis=mybir.AxisListType.X)
                rinv = rows.tile([1, 1], F32, tag="rinv", name=f"rinv_{b}")
                nc.vector.reciprocal(rinv[:], l_sb[:])
                # unpermute p_row (sb hh t q order) to natural s order
                att_row = rows.tile([1, S], F32, tag="att", name=f"att_{b}")
                pview = bs["p_row"][0:1, :].rearrange(
                    "o (sb hh t q) -> o sb q hh t",
                    sb=NBLK // 2, hh=2, t=NT, q=128)
                nc.vector.tensor_scalar_mul(att_row[:], pview, rinv[:, :1])
                nc.gpsimd.dma_start(out=att_o[b:b + 1, :], in_=att_row[:])
                ctx_sb = rows.tile([1, VD], F32, tag="ctx", name=f"ctxsb_{b}")
                nc.vector.tensor_scalar_mul(ctx_sb[:], bs["ctx_acc"][:],
                                            rinv[:, :1])
                nc.gpsimd.dma_start(out=ctx_o[b:b + 1, :], in_=ctx_sb[:])

            blocks = [(b, j) for b in range(BPC) for j in range(NBLK)]
            pending = []
            epi_queue = []  # (batch, tails_remaining_until_emit)
            done_tails = 0

            def tail_done(st):
                nonlocal done_tails
                emit_tail(st)
                done_tails += 1
                if st["j"] == NBLK - 1:
                    epi_queue.append((st["b"], done_tails + EPI_LAG))
                while epi_queue and epi_queue[0][1] <= done_tails:
                    emit_epilogue(epi_queue.pop(0)[0])

            for (b, j) in blocks:
                pending.append(emit_head(b, j))
                if len(pending) > TAIL_LAG:
                    tail_done(pending.pop(0))
            for st in pending:
                tail_done(st)
            while epi_queue:
                emit_epilogue(epi_queue.pop(0)[0])

    nc.compile()
    return nc


def _get_nc():
    if "nc" not in _CACHE:
        _CACHE["nc"] = _build()
    return _CACHE["nc"]


def run(inputs, trace=False, tmpdir=None):
    nc = _get_nc()
    value = np.ascontiguousarray(np.asarray(inputs["value"], dtype=np.float32))
    query = np.ascontiguousarray(np.asarray(inputs["query"], dtype=np.float32))
    mask = np.ascontiguousarray(np.asarray(inputs["mask"]).astype(np.uint8))
    wk = np.ascontiguousarray(np.asarray(inputs["Wk"], dtype=np.float32))
    wq = np.ascontiguousarray(np.asarray(inputs["Wq"], dtype=np.float32))
    bq = np.asarray(inputs["bq"], dtype=np.float32).reshape(1, HD)
    wo = np.asarray(inputs["Wo"], dtype=np.float32).reshape(1, HD)
    bo = np.asarray(inputs["bo"], dtype=np.float32).reshape(1, 1)

    in_maps = []
    for c in range(NCORES):
        sl = slice(c * BPC, (c + 1) * BPC)
        in_maps.append({
            "value_s": value[sl],
            "query_s": query[sl],
            "mask_s": mask[sl],
            "Wk": wk, "Wq": wq, "bq": bq, "Wo": wo, "bo": bo,
        })
    res = run_bass_kernel_spmd(nc, in_maps, core_ids=list(range(NCORES)),
                               trace=trace, tmpdir=tmpdir)
    ctx = np.concatenate([r["ctx_s"] for r in res.results], axis=0)
    att = np.concatenate([r["att_s"] for r in res.results], axis=0)
    return (ctx, att), res


def kernel(**inputs):
    (ctx, att), _ = run(inputs)
    return ctx, att


if __name__ == "__main__":
    rng = np.random.default_rng(0)
    demo = {
        "query": rng.standard_normal((B, QD)).astype(np.float32),
        "value": rng.standard_normal((B, S, VD)).astype(np.float32),
        "mask": np.ones((B, S), dtype=bool),
        "Wk": rng.uniform(-1, 1, (VD, HD)).astype(np.float32) / np.sqrt(VD),
        "Wq": rng.uniform(-1, 1, (QD, HD)).astype(np.float32) / np.sqrt(QD),
        "bq": rng.uniform(-1, 1, HD).astype(np.float32) / np.sqrt(QD),
        "Wo": rng.uniform(-1, 1, HD).astype(np.float32) / np.sqrt(HD),
        "bo": np.float32(0.01),
    }
    ctx, att = kernel(**demo)
    print("ctx", ctx.shape, "att", att.shape)


# revision 44
# speedup vs baseline: 28870.5278x; 1.0412x over previous
"""Bahdanau-style additive attention kernel for Trainium2 (8 NeuronCores).

ctx, att = attention(query, value, mask, Wk, Wq, bq, Wo, bo)
  k      = value @ Wk                          [B,S,H]
  q      = query @ Wq + bq                     [B,H]
  scores = tanh(q[:,None,:] + k) @ Wo + bo     [B,S]
  att    = softmax(mask ? scores : -1e9)       [B,S]
  ctx    = sum_s att * value                   [B,V]

Sharding: data-parallel over batch (4 batches per core, 8 cores).
Single pass over `value` per core: cast-load fp16, xbar-DMA transpose for the
k-projection (contraction dim on partitions), max-free softmax (scores are
bounded by sum|Wo|+|bo| <= ~11.4, we shift the exponent by -4 so fp16 weights
cannot overflow), and the weighted sum is accumulated from the same value
tiles so HBM traffic is one read of `value`.
"""

import os
import sys

for _r in ("/opt/trn_rl_repo", "/root/.axon_site/_ro/trn_rl_repo"):
    if os.path.isdir(_r):
        for _p in (_r, os.path.join(_r, "concourse")):
            if _p not in sys.path:
                sys.path.insert(0, _p)
        break

import numpy as np

import concourse.bacc as bacc
import concourse.mybir as mybir
from concourse import tile
from concourse.bass_utils import run_bass_kernel_spmd

B, S, QD, VD, HD = 32, 4096, 512, 512, 512
NCORES = 8
BPC = B // NCORES      # batches per core
SBLK = 512             # seq positions per block
NBLK = S // SBLK       # 8 blocks per batch
NT = SBLK // 128       # 4 seq tiles per block
NVC = VD // 128        # value-dim chunks
NHC = HD // 128        # hidden-dim chunks
NQC = QD // 128        # query-dim chunks
EXP_SHIFT = -4.0       # exp(scores + bo + EXP_SHIFT); cancels in att/ctx

F32 = mybir.dt.float32
F16 = mybir.dt.float16
U8 = mybir.dt.uint8
AF = mybir.ActivationFunctionType
ALU = mybir.AluOpType

_CACHE = {}


def _build():
    nc = bacc.Bacc(None, target_bir_lowering=False, debug=False,
                   num_swdge_queues=4)
    value = nc.dram_tensor("value_s", [BPC, S, VD], F32, kind="ExternalInput")
    query = nc.dram_tensor("query_s", [BPC, QD], F32, kind="ExternalInput")
    mask = nc.dram_tensor("mask_s", [BPC, S], U8, kind="ExternalInput")
    wk = nc.dram_tensor("Wk", [VD, HD], F32, kind="ExternalInput")
    wq = nc.dram_tensor("Wq", [QD, HD], F32, kind="ExternalInput")
    bq = nc.dram_tensor("bq", [1, HD], F32, kind="ExternalInput")
    wo = nc.dram_tensor("Wo", [1, HD], F32, kind="ExternalInput")
    bo = nc.dram_tensor("bo", [1, 1], F32, kind="ExternalInput")
    ctx_o = nc.dram_tensor("ctx_s", [BPC, VD], F32, kind="ExternalOutput")
    att_o = nc.dram_tensor("att_s", [BPC, S], F32, kind="ExternalOutput")

    TAIL_LAG = 2
    EPI_LAG = 4

    with tile.TileContext(nc) as tc:
        with tc.tile_pool(name="persist", bufs=1) as pp, \
             tc.tile_pool(name="vn", bufs=5) as vn_pool, \
             tc.tile_pool(name="vt", bufs=4) as vt_pool, \
             tc.tile_pool(name="ht", bufs=4) as ht_pool, \
             tc.tile_pool(name="rows", bufs=2) as rows, \
             tc.tile_pool(name="kt_ps", bufs=3, space="PSUM") as kt_pool, \
             tc.tile_pool(name="sc_ps", bufs=2, space="PSUM") as sc_pool, \
             tc.tile_pool(name="pt_ps", bufs=2, space="PSUM") as pt_pool, \
             tc.tile_pool(name="cx_ps", bufs=1, space="PSUM") as cx_pool:

            # ---------- superblock loader (2 seq blocks per DMA) ----------
            supers = {}

            def ensure_super(b, sbk):
                key = (b, sbk)
                if key in supers:
                    return supers[key]
                s0 = sbk * 2 * SBLK
                v_nat = vn_pool.tile([128, 2 * NT, VD], F16, tag="vn",
                                     name=f"vn_{b}_{sbk}")
                # (p t) order: partition p holds 8 consecutive seq rows, so
                # each partition reads one 16KB-contiguous DRAM run.
                # v_nat[p, t, v] = value[b, s0 + 8p + t, v]
                src = value[b, s0:s0 + 2 * SBLK, :].rearrange(
                    "(p t) v -> p t v", p=128)
                if sbk == 0:
                    # latency-critical first superblock of a batch: split per
                    # half so the first transpose starts sooner
                    for h in range(2):
                        nc.gpsimd.dma_start(
                            out=v_nat[:, 4 * h:4 * h + 4, :],
                            in_=src[:, 4 * h:4 * h + 4, :])
                else:
                    nc.gpsimd.dma_start(out=v_nat[:], in_=src)
                # vT8[p,tg,c,q] = value[b, s0 + 8q + tg, c*128+p], tg in [0,8)
                # (block j = 2*sbk+h owns tg = 4h+t; score column n = t*128+q)
                vT8 = vt_pool.tile([128, 2 * NT, NVC, 128], F16, tag="vt4",
                                   name=f"vT8_{b}_{sbk}")
                if sbk == 0:
                    # per-half transposes so the first k-matmuls start sooner
                    for h in range(2):
                        nc.sync.dma_start(
                            out=vT8[:, h * NT:(h + 1) * NT, :, :].rearrange(
                                "p t c s -> p (t c) s"),
                            in_=v_nat[:, h * NT:(h + 1) * NT, :].rearrange(
                                "p t v -> p (t v)"),
                            transpose=True)
                else:
                    nc.sync.dma_start(
                        out=vT8[:].rearrange("p t c s -> p (t c) s"),
                        in_=v_nat[:].rearrange("p t v -> p (t v)"),
                        transpose=True)
                supers[key] = (v_nat, vT8)
                return supers[key]

            # ---------- persistent constants ----------
            wk16 = pp.tile([128, NVC, HD], F16)
            ones16 = pp.tile([1, BPC], F16)
            onesf = pp.tile([1, 1], F32)
            woT16 = pp.tile([128, NHC], F16)
            bo_sb = pp.tile([1, 1], F32)
            bo4 = pp.tile([1, 1], F32)
            qt_sb = pp.tile([128, NHC, BPC], F32)
            wq16 = pp.tile([128, NQC, HD], F16)
            qT16 = pp.tile([128, NQC, BPC], F16)
            bq16 = pp.tile([1, HD], F16)
            wo16 = pp.tile([1, HD], F16)

            # prefetch the first superblock before any constant loads
            ensure_super(0, 0)

            # ---------- preamble ----------
            nc.vector.memset(ones16[:], 1.0)
            nc.vector.memset(onesf[:], 1.0)
            nc.gpsimd.dma_start(out=wo16[:], in_=wo[:, :])
            nc.gpsimd.dma_start(out=bq16[:], in_=bq[:, :])
            nc.gpsimd.dma_start(out=bo_sb[:], in_=bo[:, :])
            nc.vector.tensor_scalar_add(bo4[:], bo_sb[:], EXP_SHIFT)
            for qc in range(NQC):
                nc.gpsimd.dma_start(
                    out=qT16[:, qc, :],
                    in_=query[:, qc * 128:(qc + 1) * 128].rearrange("b p -> p b"))
            nc.gpsimd.dma_start(out=wq16[:],
                                in_=wq[:, :].rearrange("(c p) h -> p c h", p=128))
            for vc in range(NVC):
                nc.gpsimd.dma_start(out=wk16[:, vc, :],
                                    in_=wk[vc * 128:(vc + 1) * 128, :])
            woT_ps = pt_pool.tile([128, NHC], F32, tag="ptps")
            for hc in range(NHC):
                nc.tensor.matmul(woT_ps[:, hc:hc + 1],
                                 wo16[:, hc * 128:(hc + 1) * 128], ones16[:, :1])
            nc.vector.tensor_copy(woT16[:], woT_ps[:])
            for hc in range(NHC):
                qt_ps = pt_pool.tile([128, BPC], F32, tag="ptps",
                                     name=f"qt_ps_{hc}")
                for qc in range(NQC):
                    nc.tensor.matmul(qt_ps[:],
                                     wq16[:, qc, hc * 128:(hc + 1) * 128],
                                     qT16[:, qc, :],
                                     start=(qc == 0), stop=False)
                nc.tensor.matmul(qt_ps[:], bq16[:, hc * 128:(hc + 1) * 128],
                                 ones16[:, :BPC], start=False, stop=True)
                nc.vector.tensor_copy(qt_sb[:, hc, :], qt_ps[:])

            # ---------- software-pipelined main loop ----------
            batch_state = {}

            def get_batch(b):
                if b not in batch_state:
                    mask_row = rows.tile([1, S], F16, tag="mask",
                                         name=f"mask_{b}")
                    nc.gpsimd.dma_start(out=mask_row[:], in_=mask[b:b + 1, :])
                    batch_state[b] = dict(
                        mask_row=mask_row,
                        p_row=rows.tile([1, S], F32, tag="p", name=f"p_{b}"),
                        l_parts=rows.tile([1, NBLK], F32, tag="lp",
                                          name=f"lp_{b}"),
                        ctx_acc=rows.tile([1, VD], F32, tag="ctxacc",
                                          name=f"cacc_{b}"),
                    )
                return batch_state[b]

            def emit_head(b, j):
                get_batch(b)
                v_nat, vT8 = ensure_super(b, j // 2)
                h = j % 2
                hT = ht_pool.tile([128, NHC, SBLK], F16, tag="ht",
                                  name=f"hT_{b}_{j}")
                for hc in range(NHC):
                    kt_ps = kt_pool.tile([128, SBLK], F32, tag="kt",
                                         name=f"kt_{b}_{j}_{hc}")
                    for vc in range(NVC):
                        nc.tensor.matmul(kt_ps[:],
                                         wk16[:, vc, hc * 128:(hc + 1) * 128],
                                         vT8[:, h * NT:(h + 1) * NT, vc, :],
                                         start=(vc == 0), stop=(vc == NVC - 1))
                    nc.scalar.activation(hT[:, hc, :], kt_ps[:], AF.Tanh,
                                         bias=qt_sb[:, hc, b:b + 1])
                return dict(b=b, j=j, h=h, hT=hT, v_nat=v_nat)

            def emit_tail(st):
                b, j, h, hT, v_nat = st["b"], st["j"], st["h"], st["hT"], st["v_nat"]
                bs = get_batch(b)
                sj = j * SBLK
                sc_ps = sc_pool.tile([1, SBLK], F32, tag="sc",
                                     name=f"sc_{b}_{j}")
                for hc in range(NHC):
                    nc.tensor.matmul(sc_ps[:], woT16[:, hc:hc + 1], hT[:, hc, :],
                                     start=(hc == 0), stop=(hc == NHC - 1))
                e_blk = rows.tile([1, SBLK], F32, tag="eblk", name=f"e_{b}_{j}")
                nc.scalar.activation(e_blk[:], sc_ps[:], AF.Exp, bias=bo4[:, :1])
                # mask in permuted order: column n = t*128+q <-> s = s0+8q+4h+t
                s0 = (j // 2) * 2 * SBLK
                mview = bs["mask_row"][0:1, s0:s0 + 2 * SBLK].rearrange(
                    "o (q x) -> o q x", q=128)[:, :, 4 * h:4 * h + 4].rearrange(
                    "o q t -> o t q")
                nc.vector.scalar_tensor_tensor(
                    bs["p_row"][:, sj:sj + SBLK], e_blk[:], 1.0,
                    mview,
                    op0=ALU.mult, op1=ALU.mult,
                    accum_out=bs["l_parts"][:, j:j + 1])
                pt_ps = pt_pool.tile([128, NT], F32, tag="ptps",
                                     name=f"pt_{b}_{j}")
                for t in range(NT):
                    nc.tensor.matmul(pt_ps[:, t:t + 1],
                                     bs["p_row"][:, sj + t * 128:sj + (t + 1) * 128],
                                     onesf[:, :1])
                pT16 = rows.tile([128, NT], F16, tag="pt16", name=f"pt16_{b}_{j}")
                nc.vector.tensor_copy(pT16[:], pt_ps[:])
                ctx_ps = cx_pool.tile([1, VD], F32, tag="cx", name=f"cx_{b}_{j}")
                for t in range(NT):
                    nc.tensor.matmul(ctx_ps[:], pT16[:, t:t + 1],
                                     v_nat[:, h * NT + t, :],
                                     start=(t == 0), stop=(t == NT - 1))
                if j == 0:
                    nc.vector.tensor_copy(bs["ctx_acc"][:], ctx_ps[:])
                else:
                    nc.vector.tensor_tensor(bs["ctx_acc"][:], bs["ctx_acc"][:],
                                            ctx_ps[:], op=ALU.add)
            def emit_epilogue(b):
                bs = batch_state[b]
                l_sb = rows.tile([1, 1], F32, tag="l", name=f"l_{b}")
                nc.vector.reduce_sum(l_sb[:], bs["l_parts"][:],
                                     axis=mybir.AxisListType.X)
                rinv = rows.tile([1, 1], F32, tag="rinv", name=f"rinv_{b}")
                nc.vector.reciprocal(rinv[:], l_sb[:])
                # unpermute p_row (sb hh t q order) to natural s order
                att_row = rows.tile([1, S], F32, tag="att", name=f"att_{b}")
                pview = bs["p_row"][0:1, :].rearrange(
                    "o (sb hh t q) -> o sb q hh t",
                    sb=NBLK // 2, hh=2, t=NT, q=128)
                nc.vector.tensor_scalar_mul(att_row[:], pview, rinv[:, :1])
                nc.gpsimd.dma_start(out=att_o[b:b + 1, :], in_=att_row[:])
                ctx_sb = rows.tile([1, VD], F32, tag="ctx", name=f"ctxsb_{b}")
                nc.vector.tensor_scalar_mul(ctx_sb[:], bs["ctx_acc"][:],
                                            rinv[:, :1])
                nc.gpsimd.dma_start(out=ctx_o[b:b + 1, :], in_=ctx_sb[:])

            blocks = [(b, j) for b in range(BPC) for j in range(NBLK)]
            pending = []
            epi_queue = []  # (batch, tails_remaining_until_emit)
            done_tails = 0

            def tail_done(st):
                nonlocal done_tails
                emit_tail(st)
                done_tails += 1
                if st["j"] == NBLK - 1:
                    epi_queue.append((st["b"], done_tails + EPI_LAG))
                while epi_queue and epi_queue[0][1] <= done_tails:
                    emit_epilogue(epi_queue.pop(0)[0])

            for (b, j) in blocks:
                pending.append(emit_head(b, j))
                if len(pending) > TAIL_LAG:
                    tail_done(pending.pop(0))
            for st in pending:
                tail_done(st)
            while epi_queue:
                emit_epilogue(epi_queue.pop(0)[0])

    nc.compile()
    return nc


def _get_nc():
    if "nc" not in _CACHE:
        _CACHE["nc"] = _build()
    return _CACHE["nc"]


def run(inputs, trace=False, tmpdir=None):
    nc = _get_nc()
    value = np.ascontiguousarray(np.asarray(inputs["value"], dtype=np.float32))
    query = np.ascontiguousarray(np.asarray(inputs["query"], dtype=np.float32))
    mask = np.ascontiguousarray(np.asarray(inputs["mask"]).astype(np.uint8))
    wk = np.ascontiguousarray(np.asarray(inputs["Wk"], dtype=np.float32))
    wq = np.ascontiguousarray(np.asarray(inputs["Wq"], dtype=np.float32))
    bq = np.asarray(inputs["bq"], dtype=np.float32).reshape(1, HD)
    wo = np.asarray(inputs["Wo"], dtype=np.float32).reshape(1, HD)
    bo = np.asarray(inputs["bo"], dtype=np.float32).reshape(1, 1)

    in_maps = []
    for c in range(NCORES):
        sl = slice(c * BPC, (c + 1) * BPC)
        in_maps.append({
            "value_s": value[sl],
            "query_s": query[sl],
            "mask_s": mask[sl],
            "Wk": wk, "Wq": wq, "bq": bq, "Wo": wo, "bo": bo,
        })
    res = run_bass_kernel_spmd(nc, in_maps, core_ids=list(range(NCORES)),
                               trace=trace, tmpdir=tmpdir)
    ctx = np.concatenate([r["ctx_s"] for r in res.results], axis=0)
    att = np.concatenate([r["att_s"] for r in res.results], axis=0)
    return (ctx, att), res


def kernel(**inputs):
    (ctx, att), _ = run(inputs)
    return ctx, att


if __name__ == "__main__":
    rng = np.random.default_rng(0)
    demo = {
        "query": rng.standard_normal((B, QD)).astype(np.float32),
        "value": rng.standard_normal((B, S, VD)).astype(np.float32),
        "mask": np.ones((B, S), dtype=bool),
        "Wk": rng.uniform(-1, 1, (VD, HD)).astype(np.float32) / np.sqrt(VD),
        "Wq": rng.uniform(-1, 1, (QD, HD)).astype(np.float32) / np.sqrt(QD),
        "bq": rng.uniform(-1, 1, HD).astype(np.float32) / np.sqrt(QD),
        "Wo": rng.uniform(-1, 1, HD).astype(np.float32) / np.sqrt(HD),
        "bo": np.float32(0.01),
    }
    ctx, att = kernel(**demo)
    print("ctx", ctx.shape, "att", att.shape)


# revision 45
# speedup vs baseline: 29242.3403x; 1.0129x over previous
"""Bahdanau-style additive attention kernel for Trainium2 (8 NeuronCores).

ctx, att = attention(query, value, mask, Wk, Wq, bq, Wo, bo)
  k      = value @ Wk                          [B,S,H]
  q      = query @ Wq + bq                     [B,H]
  scores = tanh(q[:,None,:] + k) @ Wo + bo     [B,S]
  att    = softmax(mask ? scores : -1e9)       [B,S]
  ctx    = sum_s att * value                   [B,V]

Sharding: data-parallel over batch (4 batches per core, 8 cores).
Single pass over `value` per core: cast-load fp16, xbar-DMA transpose for the
k-projection (contraction dim on partitions), max-free softmax (scores are
bounded by sum|Wo|+|bo| <= ~11.4, we shift the exponent by -4 so fp16 weights
cannot overflow), and the weighted sum is accumulated from the same value
tiles so HBM traffic is one read of `value`.
"""

import os
import sys

for _r in ("/opt/trn_rl_repo", "/root/.axon_site/_ro/trn_rl_repo"):
    if os.path.isdir(_r):
        for _p in (_r, os.path.join(_r, "concourse")):
            if _p not in sys.path:
                sys.path.insert(0, _p)
        break

import numpy as np

import concourse.bacc as bacc
import concourse.mybir as mybir
from concourse import tile
from concourse.bass_utils import run_bass_kernel_spmd

B, S, QD, VD, HD = 32, 4096, 512, 512, 512
NCORES = 8
BPC = B // NCORES      # batches per core
SBLK = 512             # seq positions per block
NBLK = S // SBLK       # 8 blocks per batch
NT = SBLK // 128       # 4 seq tiles per block
NVC = VD // 128        # value-dim chunks
NHC = HD // 128        # hidden-dim chunks
NQC = QD // 128        # query-dim chunks
EXP_SHIFT = -4.0       # exp(scores + bo + EXP_SHIFT); cancels in att/ctx

F32 = mybir.dt.float32
F16 = mybir.dt.float16
U8 = mybir.dt.uint8
AF = mybir.ActivationFunctionType
ALU = mybir.AluOpType

_CACHE = {}


def _build():
    nc = bacc.Bacc(None, target_bir_lowering=False, debug=False,
                   num_swdge_queues=4)
    value = nc.dram_tensor("value_s", [BPC, S, VD], F32, kind="ExternalInput")
    query = nc.dram_tensor("query_s", [BPC, QD], F32, kind="ExternalInput")
    mask = nc.dram_tensor("mask_s", [BPC, S], U8, kind="ExternalInput")
    wk = nc.dram_tensor("Wk", [VD, HD], F32, kind="ExternalInput")
    wq = nc.dram_tensor("Wq", [QD, HD], F32, kind="ExternalInput")
    bq = nc.dram_tensor("bq", [1, HD], F32, kind="ExternalInput")
    wo = nc.dram_tensor("Wo", [1, HD], F32, kind="ExternalInput")
    bo = nc.dram_tensor("bo", [1, 1], F32, kind="ExternalInput")
    ctx_o = nc.dram_tensor("ctx_s", [BPC, VD], F32, kind="ExternalOutput")
    att_o = nc.dram_tensor("att_s", [BPC, S], F32, kind="ExternalOutput")

    TAIL_LAG = 2
    EPI_LAG = 4

    with tile.TileContext(nc) as tc:
        with tc.tile_pool(name="persist", bufs=1) as pp, \
             tc.tile_pool(name="vn", bufs=5) as vn_pool, \
             tc.tile_pool(name="vt", bufs=4) as vt_pool, \
             tc.tile_pool(name="ht", bufs=4) as ht_pool, \
             tc.tile_pool(name="rows", bufs=2) as rows, \
             tc.tile_pool(name="kt_ps", bufs=4, space="PSUM") as kt_pool, \
             tc.tile_pool(name="sc_ps", bufs=2, space="PSUM") as sc_pool, \
             tc.tile_pool(name="pt_ps", bufs=1, space="PSUM") as pt_pool, \
             tc.tile_pool(name="cx_ps", bufs=1, space="PSUM") as cx_pool:

            # ---------- superblock loader (2 seq blocks per DMA) ----------
            supers = {}

            def ensure_super(b, sbk):
                key = (b, sbk)
                if key in supers:
                    return supers[key]
                s0 = sbk * 2 * SBLK
                v_nat = vn_pool.tile([128, 2 * NT, VD], F16, tag="vn",
                                     name=f"vn_{b}_{sbk}")
                # (p t) order: partition p holds 8 consecutive seq rows, so
                # each partition reads one 16KB-contiguous DRAM run.
                # v_nat[p, t, v] = value[b, s0 + 8p + t, v]
                src = value[b, s0:s0 + 2 * SBLK, :].rearrange(
                    "(p t) v -> p t v", p=128)
                if sbk == 0:
                    # latency-critical first superblock of a batch: split per
                    # half so the first transpose starts sooner
                    for h in range(2):
                        nc.gpsimd.dma_start(
                            out=v_nat[:, 4 * h:4 * h + 4, :],
                            in_=src[:, 4 * h:4 * h + 4, :])
                else:
                    nc.gpsimd.dma_start(out=v_nat[:], in_=src)
                # vT8[p,tg,c,q] = value[b, s0 + 8q + tg, c*128+p], tg in [0,8)
                # (block j = 2*sbk+h owns tg = 4h+t; score column n = t*128+q)
                vT8 = vt_pool.tile([128, 2 * NT, NVC, 128], F16, tag="vt4",
                                   name=f"vT8_{b}_{sbk}")
                if sbk == 0:
                    # per-half transposes so the first k-matmuls start sooner
                    for h in range(2):
                        nc.sync.dma_start(
                            out=vT8[:, h * NT:(h + 1) * NT, :, :].rearrange(
                                "p t c s -> p (t c) s"),
                            in_=v_nat[:, h * NT:(h + 1) * NT, :].rearrange(
                                "p t v -> p (t v)"),
                            transpose=True)
                else:
                    nc.sync.dma_start(
                        out=vT8[:].rearrange("p t c s -> p (t c) s"),
                        in_=v_nat[:].rearrange("p t v -> p (t v)"),
                        transpose=True)
                supers[key] = (v_nat, vT8)
                return supers[key]

            # ---------- persistent constants ----------
            wk16 = pp.tile([128, NVC, HD], F16)
            ones16 = pp.tile([1, BPC], F16)
            onesf = pp.tile([1, 1], F32)
            woT16 = pp.tile([128, NHC], F16)
            bo_sb = pp.tile([1, 1], F32)
            bo4 = pp.tile([1, 1], F32)
            qt_sb = pp.tile([128, NHC, BPC], F32)
            wq16 = pp.tile([128, NQC, HD], F16)
            qT16 = pp.tile([128, NQC, BPC], F16)
            bq16 = pp.tile([1, HD], F16)
            wo16 = pp.tile([1, HD], F16)

            # prefetch the first superblock before any constant loads
            ensure_super(0, 0)

            # ---------- preamble ----------
            nc.vector.memset(ones16[:], 1.0)
            nc.vector.memset(onesf[:], 1.0)
            nc.gpsimd.dma_start(out=wo16[:], in_=wo[:, :])
            nc.gpsimd.dma_start(out=bq16[:], in_=bq[:, :])
            nc.gpsimd.dma_start(out=bo_sb[:], in_=bo[:, :])
            nc.vector.tensor_scalar_add(bo4[:], bo_sb[:], EXP_SHIFT)
            for qc in range(NQC):
                nc.gpsimd.dma_start(
                    out=qT16[:, qc, :],
                    in_=query[:, qc * 128:(qc + 1) * 128].rearrange("b p -> p b"))
            nc.gpsimd.dma_start(out=wq16[:],
                                in_=wq[:, :].rearrange("(c p) h -> p c h", p=128))
            for vc in range(NVC):
                nc.gpsimd.dma_start(out=wk16[:, vc, :],
                                    in_=wk[vc * 128:(vc + 1) * 128, :])
            woT_ps = pt_pool.tile([128, NHC], F32, tag="ptps")
            for hc in range(NHC):
                nc.tensor.matmul(woT_ps[:, hc:hc + 1],
                                 wo16[:, hc * 128:(hc + 1) * 128], ones16[:, :1])
            nc.vector.tensor_copy(woT16[:], woT_ps[:])
            for hc in range(NHC):
                qt_ps = pt_pool.tile([128, BPC], F32, tag="ptps",
                                     name=f"qt_ps_{hc}")
                for qc in range(NQC):
                    nc.tensor.matmul(qt_ps[:],
                                     wq16[:, qc, hc * 128:(hc + 1) * 128],
                                     qT16[:, qc, :],
                                     start=(qc == 0), stop=False)
                nc.tensor.matmul(qt_ps[:], bq16[:, hc * 128:(hc + 1) * 128],
                                 ones16[:, :BPC], start=False, stop=True)
                nc.vector.tensor_copy(qt_sb[:, hc, :], qt_ps[:])

            # ---------- software-pipelined main loop ----------
            batch_state = {}

            def get_batch(b):
                if b not in batch_state:
                    mask_row = rows.tile([1, S], F16, tag="mask",
                                         name=f"mask_{b}")
                    nc.gpsimd.dma_start(out=mask_row[:], in_=mask[b:b + 1, :])
                    batch_state[b] = dict(
                        mask_row=mask_row,
                        p_row=rows.tile([1, S], F32, tag="p", name=f"p_{b}"),
                        l_parts=rows.tile([1, NBLK], F32, tag="lp",
                                          name=f"lp_{b}"),
                        ctx_acc=rows.tile([1, VD], F32, tag="ctxacc",
                                          name=f"cacc_{b}"),
                    )
                return batch_state[b]

            def emit_head(b, j):
                get_batch(b)
                v_nat, vT8 = ensure_super(b, j // 2)
                h = j % 2
                hT = ht_pool.tile([128, NHC, SBLK], F16, tag="ht",
                                  name=f"hT_{b}_{j}")
                for hc in range(NHC):
                    kt_ps = kt_pool.tile([128, SBLK], F32, tag="kt",
                                         name=f"kt_{b}_{j}_{hc}")
                    for vc in range(NVC):
                        nc.tensor.matmul(kt_ps[:],
                                         wk16[:, vc, hc * 128:(hc + 1) * 128],
                                         vT8[:, h * NT:(h + 1) * NT, vc, :],
                                         start=(vc == 0), stop=(vc == NVC - 1))
                    nc.scalar.activation(hT[:, hc, :], kt_ps[:], AF.Tanh,
                                         bias=qt_sb[:, hc, b:b + 1])
                return dict(b=b, j=j, h=h, hT=hT, v_nat=v_nat)

            def emit_tail(st):
                b, j, h, hT, v_nat = st["b"], st["j"], st["h"], st["hT"], st["v_nat"]
                bs = get_batch(b)
                sj = j * SBLK
                sc_ps = sc_pool.tile([1, SBLK], F32, tag="sc",
                                     name=f"sc_{b}_{j}")
                for hc in range(NHC):
                    nc.tensor.matmul(sc_ps[:], woT16[:, hc:hc + 1], hT[:, hc, :],
                                     start=(hc == 0), stop=(hc == NHC - 1))
                e_blk = rows.tile([1, SBLK], F32, tag="eblk", name=f"e_{b}_{j}")
                nc.scalar.activation(e_blk[:], sc_ps[:], AF.Exp, bias=bo4[:, :1])
                # mask in permuted order: column n = t*128+q <-> s = s0+8q+4h+t
                s0 = (j // 2) * 2 * SBLK
                mview = bs["mask_row"][0:1, s0:s0 + 2 * SBLK].rearrange(
                    "o (q x) -> o q x", q=128)[:, :, 4 * h:4 * h + 4].rearrange(
                    "o q t -> o t q")
                nc.vector.scalar_tensor_tensor(
                    bs["p_row"][:, sj:sj + SBLK], e_blk[:], 1.0,
                    mview,
                    op0=ALU.mult, op1=ALU.mult,
                    accum_out=bs["l_parts"][:, j:j + 1])
                pt_ps = pt_pool.tile([128, NT], F32, tag="ptps",
                                     name=f"pt_{b}_{j}")
                for t in range(NT):
                    nc.tensor.matmul(pt_ps[:, t:t + 1],
                                     bs["p_row"][:, sj + t * 128:sj + (t + 1) * 128],
                                     onesf[:, :1])
                pT16 = rows.tile([128, NT], F16, tag="pt16", name=f"pt16_{b}_{j}")
                nc.vector.tensor_copy(pT16[:], pt_ps[:])
                ctx_ps = cx_pool.tile([1, VD], F32, tag="cx", name=f"cx_{b}_{j}")
                for t in range(NT):
                    nc.tensor.matmul(ctx_ps[:], pT16[:, t:t + 1],
                                     v_nat[:, h * NT + t, :],
                                     start=(t == 0), stop=(t == NT - 1))
                if j == 0:
                    nc.vector.tensor_copy(bs["ctx_acc"][:], ctx_ps[:])
                else:
                    nc.vector.tensor_tensor(bs["ctx_acc"][:], bs["ctx_acc"][:],
                                            ctx_ps[:], op=ALU.add)
            def emit_epilogue(b):
                bs = batch_state[b]
                l_sb = rows.tile([1, 1], F32, tag="l", name=f"l_{b}")
                nc.vector.reduce_sum(l_sb[:], bs["l_parts"][:],
                                     axis=mybir.AxisListType.X)
                rinv = rows.tile([1, 1], F32, tag="rinv", name=f"rinv_{b}")
                nc.vector.reciprocal(rinv[:], l_sb[:])
                # unpermute p_row (sb hh t q order) to natural s order
                att_row = rows.tile([1, S], F32, tag="att", name=f"att_{b}")
                pview = bs["p_row"][0:1, :].rearrange(
                    "o (sb hh t q) -> o sb q hh t",
                    sb=NBLK // 2, hh=2, t=NT, q=128)
                nc.vector.tensor_scalar_mul(att_row[:], pview, rinv[:, :1])
                nc.gpsimd.dma_start(out=att_o[b:b + 1, :], in_=att_row[:])
                ctx_sb = rows.tile([1, VD], F32, tag="ctx", name=f"ctxsb_{b}")
                nc.vector.tensor_scalar_mul(ctx_sb[:], bs["ctx_acc"][:],
                                            rinv[:, :1])
                nc.gpsimd.dma_start(out=ctx_o[b:b + 1, :], in_=ctx_sb[:])

            blocks = [(b, j) for b in range(BPC) for j in range(NBLK)]
            pending = []
            epi_queue = []  # (batch, tails_remaining_until_emit)
            done_tails = 0

            def tail_done(st):
                nonlocal done_tails
                emit_tail(st)
                done_tails += 1
                if st["j"] == NBLK - 1:
                    epi_queue.append((st["b"], done_tails + EPI_LAG))
                while epi_queue and epi_queue[0][1] <= done_tails:
                    emit_epilogue(epi_queue.pop(0)[0])

            for (b, j) in blocks:
                pending.append(emit_head(b, j))
                if len(pending) > TAIL_LAG:
                    tail_done(pending.pop(0))
            for st in pending:
                tail_done(st)
            while epi_queue:
                emit_epilogue(epi_queue.pop(0)[0])

    nc.compile()
    return nc


def _get_nc():
    if "nc" not in _CACHE:
        _CACHE["nc"] = _build()
    return _CACHE["nc"]


def run(inputs, trace=False, tmpdir=None):
    nc = _get_nc()
    value = np.ascontiguousarray(np.asarray(inputs["value"], dtype=np.float32))
    query = np.ascontiguousarray(np.asarray(inputs["query"], dtype=np.float32))
    mask = np.ascontiguousarray(np.asarray(inputs["mask"]).astype(np.uint8))
    wk = np.ascontiguousarray(np.asarray(inputs["Wk"], dtype=np.float32))
    wq = np.ascontiguousarray(np.asarray(inputs["Wq"], dtype=np.float32))
    bq = np.asarray(inputs["bq"], dtype=np.float32).reshape(1, HD)
    wo = np.asarray(inputs["Wo"], dtype=np.float32).reshape(1, HD)
    bo = np.asarray(inputs["bo"], dtype=np.float32).reshape(1, 1)

    in_maps = []
    for c in range(NCORES):
        sl = slice(c * BPC, (c + 1) * BPC)
        in_maps.append({
            "value_s": value[sl],
            "query_s": query[sl],
            "mask_s": mask[sl],
            "Wk": wk, "Wq": wq, "bq": bq, "Wo": wo, "bo": bo,
        })
    res = run_bass_kernel_spmd(nc, in_maps, core_ids=list(range(NCORES)),
                               trace=trace, tmpdir=tmpdir)
    ctx = np.concatenate([r["ctx_s"] for r in res.results], axis=0)
    att = np.concatenate([r["att_s"] for r in res.results], axis=0)
    return (ctx, att), res


def kernel(**inputs):
    (ctx, att), _ = run(inputs)
    return ctx, att


if __name__ == "__main__":
    rng = np.random.default_rng(0)
    demo = {
        "query": rng.standard_normal((B, QD)).astype(np.float32),
        "value": rng.standard_normal((B, S, VD)).astype(np.float32),
        "mask": np.ones((B, S), dtype=bool),
        "Wk": rng.uniform(-1, 1, (VD, HD)).astype(np.float32) / np.sqrt(VD),
        "Wq": rng.uniform(-1, 1, (QD, HD)).astype(np.float32) / np.sqrt(QD),
        "bq": rng.uniform(-1, 1, HD).astype(np.float32) / np.sqrt(QD),
        "Wo": rng.uniform(-1, 1, HD).astype(np.float32) / np.sqrt(HD),
        "bo": np.float32(0.01),
    }
    ctx, att = kernel(**demo)
    print("ctx", ctx.shape, "att", att.shape)


# revision 46
# speedup vs baseline: 29407.4659x; 1.0056x over previous
"""Bahdanau-style additive attention kernel for Trainium2 (8 NeuronCores).

ctx, att = attention(query, value, mask, Wk, Wq, bq, Wo, bo)
  k      = value @ Wk                          [B,S,H]
  q      = query @ Wq + bq                     [B,H]
  scores = tanh(q[:,None,:] + k) @ Wo + bo     [B,S]
  att    = softmax(mask ? scores : -1e9)       [B,S]
  ctx    = sum_s att * value                   [B,V]

Sharding: data-parallel over batch (4 batches per core, 8 cores).
Single pass over `value` per core: cast-load fp16, xbar-DMA transpose for the
k-projection (contraction dim on partitions), max-free softmax (scores are
bounded by sum|Wo|+|bo| <= ~11.4, we shift the exponent by -4 so fp16 weights
cannot overflow), and the weighted sum is accumulated from the same value
tiles so HBM traffic is one read of `value`.
"""

import os
import sys

for _r in ("/opt/trn_rl_repo", "/root/.axon_site/_ro/trn_rl_repo"):
    if os.path.isdir(_r):
        for _p in (_r, os.path.join(_r, "concourse")):
            if _p not in sys.path:
                sys.path.insert(0, _p)
        break

import numpy as np

import concourse.bacc as bacc
import concourse.mybir as mybir
from concourse import tile
from concourse.bass_utils import run_bass_kernel_spmd

B, S, QD, VD, HD = 32, 4096, 512, 512, 512
NCORES = 8
BPC = B // NCORES      # batches per core
SBLK = 512             # seq positions per block
NBLK = S // SBLK       # 8 blocks per batch
NT = SBLK // 128       # 4 seq tiles per block
NVC = VD // 128        # value-dim chunks
NHC = HD // 128        # hidden-dim chunks
NQC = QD // 128        # query-dim chunks
EXP_SHIFT = -4.0       # exp(scores + bo + EXP_SHIFT); cancels in att/ctx

F32 = mybir.dt.float32
F16 = mybir.dt.float16
U8 = mybir.dt.uint8
AF = mybir.ActivationFunctionType
ALU = mybir.AluOpType

_CACHE = {}


def _build():
    nc = bacc.Bacc(None, target_bir_lowering=False, debug=False,
                   num_swdge_queues=4)
    value = nc.dram_tensor("value_s", [BPC, S, VD], F32, kind="ExternalInput")
    query = nc.dram_tensor("query_s", [BPC, QD], F32, kind="ExternalInput")
    mask = nc.dram_tensor("mask_s", [BPC, S], U8, kind="ExternalInput")
    wk = nc.dram_tensor("Wk", [VD, HD], F32, kind="ExternalInput")
    wq = nc.dram_tensor("Wq", [QD, HD], F32, kind="ExternalInput")
    bq = nc.dram_tensor("bq", [1, HD], F32, kind="ExternalInput")
    wo = nc.dram_tensor("Wo", [1, HD], F32, kind="ExternalInput")
    bo = nc.dram_tensor("bo", [1, 1], F32, kind="ExternalInput")
    ctx_o = nc.dram_tensor("ctx_s", [BPC, VD], F32, kind="ExternalOutput")
    att_o = nc.dram_tensor("att_s", [BPC, S], F32, kind="ExternalOutput")

    TAIL_LAG = 2
    EPI_LAG = 4

    with tile.TileContext(nc) as tc:
        with tc.tile_pool(name="persist", bufs=1) as pp, \
             tc.tile_pool(name="vn", bufs=5) as vn_pool, \
             tc.tile_pool(name="vt", bufs=4) as vt_pool, \
             tc.tile_pool(name="ht", bufs=5) as ht_pool, \
             tc.tile_pool(name="rows", bufs=2) as rows, \
             tc.tile_pool(name="kt_ps", bufs=4, space="PSUM") as kt_pool, \
             tc.tile_pool(name="sc_ps", bufs=2, space="PSUM") as sc_pool, \
             tc.tile_pool(name="pt_ps", bufs=1, space="PSUM") as pt_pool, \
             tc.tile_pool(name="cx_ps", bufs=1, space="PSUM") as cx_pool:

            # ---------- superblock loader (2 seq blocks per DMA) ----------
            supers = {}

            def ensure_super(b, sbk):
                key = (b, sbk)
                if key in supers:
                    return supers[key]
                s0 = sbk * 2 * SBLK
                v_nat = vn_pool.tile([128, 2 * NT, VD], F16, tag="vn",
                                     name=f"vn_{b}_{sbk}")
                # (p t) order: partition p holds 8 consecutive seq rows, so
                # each partition reads one 16KB-contiguous DRAM run.
                # v_nat[p, t, v] = value[b, s0 + 8p + t, v]
                src = value[b, s0:s0 + 2 * SBLK, :].rearrange(
                    "(p t) v -> p t v", p=128)
                if sbk == 0:
                    # latency-critical first superblock of a batch: split per
                    # half so the first transpose starts sooner
                    for h in range(2):
                        nc.gpsimd.dma_start(
                            out=v_nat[:, 4 * h:4 * h + 4, :],
                            in_=src[:, 4 * h:4 * h + 4, :])
                else:
                    nc.gpsimd.dma_start(out=v_nat[:], in_=src)
                # vT8[p,tg,c,q] = value[b, s0 + 8q + tg, c*128+p], tg in [0,8)
                # (block j = 2*sbk+h owns tg = 4h+t; score column n = t*128+q)
                vT8 = vt_pool.tile([128, 2 * NT, NVC, 128], F16, tag="vt4",
                                   name=f"vT8_{b}_{sbk}")
                if sbk == 0:
                    # per-half transposes so the first k-matmuls start sooner
                    for h in range(2):
                        nc.sync.dma_start(
                            out=vT8[:, h * NT:(h + 1) * NT, :, :].rearrange(
                                "p t c s -> p (t c) s"),
                            in_=v_nat[:, h * NT:(h + 1) * NT, :].rearrange(
                                "p t v -> p (t v)"),
                            transpose=True)
                else:
                    nc.sync.dma_start(
                        out=vT8[:].rearrange("p t c s -> p (t c) s"),
                        in_=v_nat[:].rearrange("p t v -> p (t v)"),
                        transpose=True)
                supers[key] = (v_nat, vT8)
                return supers[key]

            # ---------- persistent constants ----------
            wk16 = pp.tile([128, NVC, HD], F16)
            ones16 = pp.tile([1, BPC], F16)
            onesf = pp.tile([1, 1], F32)
            woT16 = pp.tile([128, NHC], F16)
            bo_sb = pp.tile([1, 1], F32)
            bo4 = pp.tile([1, 1], F32)
            qt_sb = pp.tile([128, NHC, BPC], F32)
            wq16 = pp.tile([128, NQC, HD], F16)
            qT16 = pp.tile([128, NQC, BPC], F16)
            bq16 = pp.tile([1, HD], F16)
            wo16 = pp.tile([1, HD], F16)

            # prefetch the first superblock before any constant loads
            ensure_super(0, 0)

            # ---------- preamble ----------
            nc.vector.memset(ones16[:], 1.0)
            nc.vector.memset(onesf[:], 1.0)
            nc.gpsimd.dma_start(out=wo16[:], in_=wo[:, :])
            nc.gpsimd.dma_start(out=bq16[:], in_=bq[:, :])
            nc.gpsimd.dma_start(out=bo_sb[:], in_=bo[:, :])
            nc.vector.tensor_scalar_add(bo4[:], bo_sb[:], EXP_SHIFT)
            for qc in range(NQC):
                nc.gpsimd.dma_start(
                    out=qT16[:, qc, :],
                    in_=query[:, qc * 128:(qc + 1) * 128].rearrange("b p -> p b"))
            nc.gpsimd.dma_start(out=wq16[:],
                                in_=wq[:, :].rearrange("(c p) h -> p c h", p=128))
            for vc in range(NVC):
                nc.gpsimd.dma_start(out=wk16[:, vc, :],
                                    in_=wk[vc * 128:(vc + 1) * 128, :])
            woT_ps = pt_pool.tile([128, NHC], F32, tag="ptps")
            for hc in range(NHC):
                nc.tensor.matmul(woT_ps[:, hc:hc + 1],
                                 wo16[:, hc * 128:(hc + 1) * 128], ones16[:, :1])
            nc.vector.tensor_copy(woT16[:], woT_ps[:])
            for hc in range(NHC):
                qt_ps = pt_pool.tile([128, BPC], F32, tag="ptps",
                                     name=f"qt_ps_{hc}")
                for qc in range(NQC):
                    nc.tensor.matmul(qt_ps[:],
                                     wq16[:, qc, hc * 128:(hc + 1) * 128],
                                     qT16[:, qc, :],
                                     start=(qc == 0), stop=False)
                nc.tensor.matmul(qt_ps[:], bq16[:, hc * 128:(hc + 1) * 128],
                                 ones16[:, :BPC], start=False, stop=True)
                nc.vector.tensor_copy(qt_sb[:, hc, :], qt_ps[:])

            # ---------- software-pipelined main loop ----------
            batch_state = {}

            def get_batch(b):
                if b not in batch_state:
                    mask_row = rows.tile([1, S], F16, tag="mask",
                                         name=f"mask_{b}")
                    nc.gpsimd.dma_start(out=mask_row[:], in_=mask[b:b + 1, :])
                    batch_state[b] = dict(
                        mask_row=mask_row,
                        p_row=rows.tile([1, S], F32, tag="p", name=f"p_{b}"),
                        l_parts=rows.tile([1, NBLK], F32, tag="lp",
                                          name=f"lp_{b}"),
                        ctx_acc=rows.tile([1, VD], F32, tag="ctxacc",
                                          name=f"cacc_{b}"),
                    )
                return batch_state[b]

            def emit_head(b, j):
                get_batch(b)
                v_nat, vT8 = ensure_super(b, j // 2)
                h = j % 2
                hT = ht_pool.tile([128, NHC, SBLK], F16, tag="ht",
                                  name=f"hT_{b}_{j}")
                for hc in range(NHC):
                    kt_ps = kt_pool.tile([128, SBLK], F32, tag="kt",
                                         name=f"kt_{b}_{j}_{hc}")
                    for vc in range(NVC):
                        nc.tensor.matmul(kt_ps[:],
                                         wk16[:, vc, hc * 128:(hc + 1) * 128],
                                         vT8[:, h * NT:(h + 1) * NT, vc, :],
                                         start=(vc == 0), stop=(vc == NVC - 1))
                    nc.scalar.activation(hT[:, hc, :], kt_ps[:], AF.Tanh,
                                         bias=qt_sb[:, hc, b:b + 1])
                return dict(b=b, j=j, h=h, hT=hT, v_nat=v_nat)

            def emit_tail(st):
                b, j, h, hT, v_nat = st["b"], st["j"], st["h"], st["hT"], st["v_nat"]
                bs = get_batch(b)
                sj = j * SBLK
                sc_ps = sc_pool.tile([1, SBLK], F32, tag="sc",
                                     name=f"sc_{b}_{j}")
                for hc in range(NHC):
                    nc.tensor.matmul(sc_ps[:], woT16[:, hc:hc + 1], hT[:, hc, :],
                                     start=(hc == 0), stop=(hc == NHC - 1))
                e_blk = rows.tile([1, SBLK], F32, tag="eblk", name=f"e_{b}_{j}")
                nc.scalar.activation(e_blk[:], sc_ps[:], AF.Exp, bias=bo4[:, :1])
                # mask in permuted order: column n = t*128+q <-> s = s0+8q+4h+t
                s0 = (j // 2) * 2 * SBLK
                mview = bs["mask_row"][0:1, s0:s0 + 2 * SBLK].rearrange(
                    "o (q x) -> o q x", q=128)[:, :, 4 * h:4 * h + 4].rearrange(
                    "o q t -> o t q")
                nc.vector.scalar_tensor_tensor(
                    bs["p_row"][:, sj:sj + SBLK], e_blk[:], 1.0,
                    mview,
                    op0=ALU.mult, op1=ALU.mult,
                    accum_out=bs["l_parts"][:, j:j + 1])
                pt_ps = pt_pool.tile([128, NT], F32, tag="ptps",
                                     name=f"pt_{b}_{j}")
                for t in range(NT):
                    nc.tensor.matmul(pt_ps[:, t:t + 1],
                                     bs["p_row"][:, sj + t * 128:sj + (t + 1) * 128],
                                     onesf[:, :1])
                pT16 = rows.tile([128, NT], F16, tag="pt16", name=f"pt16_{b}_{j}")
                nc.vector.tensor_copy(pT16[:], pt_ps[:])
                ctx_ps = cx_pool.tile([1, VD], F32, tag="cx", name=f"cx_{b}_{j}")
                for t in range(NT):
                    nc.tensor.matmul(ctx_ps[:], pT16[:, t:t + 1],
                                     v_nat[:, h * NT + t, :],
                                     start=(t == 0), stop=(t == NT - 1))
                if j == 0:
                    nc.vector.tensor_copy(bs["ctx_acc"][:], ctx_ps[:])
                else:
                    nc.vector.tensor_tensor(bs["ctx_acc"][:], bs["ctx_acc"][:],
                                            ctx_ps[:], op=ALU.add)
            def emit_epilogue(b):
                bs = batch_state[b]
                l_sb = rows.tile([1, 1], F32, tag="l", name=f"l_{b}")
                nc.vector.reduce_sum(l_sb[:], bs["l_parts"][:],
                                     axis=mybir.AxisListType.X)
                rinv = rows.tile([1, 1], F32, tag="rinv", name=f"rinv_{b}")
                nc.vector.reciprocal(rinv[:], l_sb[:])
                # unpermute p_row (sb hh t q order) to natural s order
                att_row = rows.tile([1, S], F32, tag="att", name=f"att_{b}")
                pview = bs["p_row"][0:1, :].rearrange(
                    "o (sb hh t q) -> o sb q hh t",
                    sb=NBLK // 2, hh=2, t=NT, q=128)
                nc.vector.tensor_scalar_mul(att_row[:], pview, rinv[:, :1])
                nc.gpsimd.dma_start(out=att_o[b:b + 1, :], in_=att_row[:])
                ctx_sb = rows.tile([1, VD], F32, tag="ctx", name=f"ctxsb_{b}")
                nc.vector.tensor_scalar_mul(ctx_sb[:], bs["ctx_acc"][:],
                                            rinv[:, :1])
                nc.gpsimd.dma_start(out=ctx_o[b:b + 1, :], in_=ctx_sb[:])

            blocks = [(b, j) for b in range(BPC) for j in range(NBLK)]
            pending = []
            epi_queue = []  # (batch, tails_remaining_until_emit)
            done_tails = 0

            def tail_done(st):
                nonlocal done_tails
                emit_tail(st)
                done_tails += 1
                if st["j"] == NBLK - 1:
                    epi_queue.append((st["b"], done_tails + EPI_LAG))
                while epi_queue and epi_queue[0][1] <= done_tails:
                    emit_epilogue(epi_queue.pop(0)[0])

            for (b, j) in blocks:
                pending.append(emit_head(b, j))
                if len(pending) > TAIL_LAG:
                    tail_done(pending.pop(0))
            for st in pending:
                tail_done(st)
            while epi_queue:
                emit_epilogue(epi_queue.pop(0)[0])

    nc.compile()
    return nc


def _get_nc():
    if "nc" not in _CACHE:
        _CACHE["nc"] = _build()
    return _CACHE["nc"]


def run(inputs, trace=False, tmpdir=None):
    nc = _get_nc()
    value = np.ascontiguousarray(np.asarray(inputs["value"], dtype=np.float32))
    query = np.ascontiguousarray(np.asarray(inputs["query"], dtype=np.float32))
    mask = np.ascontiguousarray(np.asarray(inputs["mask"]).astype(np.uint8))
    wk = np.ascontiguousarray(np.asarray(inputs["Wk"], dtype=np.float32))
    wq = np.ascontiguousarray(np.asarray(inputs["Wq"], dtype=np.float32))
    bq = np.asarray(inputs["bq"], dtype=np.float32).reshape(1, HD)
    wo = np.asarray(inputs["Wo"], dtype=np.float32).reshape(1, HD)
    bo = np.asarray(inputs["bo"], dtype=np.float32).reshape(1, 1)

    in_maps = []
    for c in range(NCORES):
        sl = slice(c * BPC, (c + 1) * BPC)
        in_maps.append({
            "value_s": value[sl],
            "query_s": query[sl],
            "mask_s": mask[sl],
            "Wk": wk, "Wq": wq, "bq": bq, "Wo": wo, "bo": bo,
        })
    res = run_bass_kernel_spmd(nc, in_maps, core_ids=list(range(NCORES)),
                               trace=trace, tmpdir=tmpdir)
    ctx = np.concatenate([r["ctx_s"] for r in res.results], axis=0)
    att = np.concatenate([r["att_s"] for r in res.results], axis=0)
    return (ctx, att), res


def kernel(**inputs):
    (ctx, att), _ = run(inputs)
    return ctx, att


if __name__ == "__main__":
    rng = np.random.default_rng(0)
    demo = {
        "query": rng.standard_normal((B, QD)).astype(np.float32),
        "value": rng.standard_normal((B, S, VD)).astype(np.float32),
        "mask": np.ones((B, S), dtype=bool),
        "Wk": rng.uniform(-1, 1, (VD, HD)).astype(np.float32) / np.sqrt(VD),
        "Wq": rng.uniform(-1, 1, (QD, HD)).astype(np.float32) / np.sqrt(QD),
        "bq": rng.uniform(-1, 1, HD).astype(np.float32) / np.sqrt(QD),
        "Wo": rng.uniform(-1, 1, HD).astype(np.float32) / np.sqrt(HD),
        "bo": np.float32(0.01),
    }
    ctx, att = kernel(**demo)
    print("ctx", ctx.shape, "att", att.shape)


# revision 47
# speedup vs baseline: 29481.4015x; 1.0025x over previous
"""Bahdanau-style additive attention kernel for Trainium2 (8 NeuronCores).

ctx, att = attention(query, value, mask, Wk, Wq, bq, Wo, bo)
  k      = value @ Wk                          [B,S,H]
  q      = query @ Wq + bq                     [B,H]
  scores = tanh(q[:,None,:] + k) @ Wo + bo     [B,S]
  att    = softmax(mask ? scores : -1e9)       [B,S]
  ctx    = sum_s att * value                   [B,V]

Sharding: data-parallel over batch (4 batches per core, 8 cores).
Single pass over `value` per core: cast-load fp16, xbar-DMA transpose for the
k-projection (contraction dim on partitions), max-free softmax (scores are
bounded by sum|Wo|+|bo| <= ~11.4, we shift the exponent by -4 so fp16 weights
cannot overflow), and the weighted sum is accumulated from the same value
tiles so HBM traffic is one read of `value`.
"""

import os
import sys

for _r in ("/opt/trn_rl_repo", "/root/.axon_site/_ro/trn_rl_repo"):
    if os.path.isdir(_r):
        for _p in (_r, os.path.join(_r, "concourse")):
            if _p not in sys.path:
                sys.path.insert(0, _p)
        break

import numpy as np

import concourse.bacc as bacc
import concourse.mybir as mybir
from concourse import tile
from concourse.bass_utils import run_bass_kernel_spmd

B, S, QD, VD, HD = 32, 4096, 512, 512, 512
NCORES = 8
BPC = B // NCORES      # batches per core
SBLK = 512             # seq positions per block
NBLK = S // SBLK       # 8 blocks per batch
NT = SBLK // 128       # 4 seq tiles per block
NVC = VD // 128        # value-dim chunks
NHC = HD // 128        # hidden-dim chunks
NQC = QD // 128        # query-dim chunks
EXP_SHIFT = -4.0       # exp(scores + bo + EXP_SHIFT); cancels in att/ctx

F32 = mybir.dt.float32
F16 = mybir.dt.float16
U8 = mybir.dt.uint8
AF = mybir.ActivationFunctionType
ALU = mybir.AluOpType

_CACHE = {}


def _build():
    nc = bacc.Bacc(None, target_bir_lowering=False, debug=False,
                   num_swdge_queues=4)
    value = nc.dram_tensor("value_s", [BPC, S, VD], F32, kind="ExternalInput")
    query = nc.dram_tensor("query_s", [BPC, QD], F32, kind="ExternalInput")
    mask = nc.dram_tensor("mask_s", [BPC, S], U8, kind="ExternalInput")
    wk = nc.dram_tensor("Wk", [VD, HD], F32, kind="ExternalInput")
    wq = nc.dram_tensor("Wq", [QD, HD], F32, kind="ExternalInput")
    bq = nc.dram_tensor("bq", [1, HD], F32, kind="ExternalInput")
    wo = nc.dram_tensor("Wo", [1, HD], F32, kind="ExternalInput")
    bo = nc.dram_tensor("bo", [1, 1], F32, kind="ExternalInput")
    ctx_o = nc.dram_tensor("ctx_s", [BPC, VD], F32, kind="ExternalOutput")
    att_o = nc.dram_tensor("att_s", [BPC, S], F32, kind="ExternalOutput")

    TAIL_LAG = 2
    EPI_LAG = 4

    from concourse.tile import add_dep_helper

    def _dep(waiter, prereq):
        for a, b2 in ((waiter, prereq), (getattr(waiter, "ins", waiter),
                                         getattr(prereq, "ins", prereq))):
            try:
                add_dep_helper(a, b2, reason="startup: stagger bulk loads")
                return
            except TypeError:
                continue

    with tile.TileContext(nc) as tc:
        with tc.tile_pool(name="persist", bufs=1) as pp, \
             tc.tile_pool(name="vn", bufs=5) as vn_pool, \
             tc.tile_pool(name="vt", bufs=4) as vt_pool, \
             tc.tile_pool(name="ht", bufs=5) as ht_pool, \
             tc.tile_pool(name="rows", bufs=2) as rows, \
             tc.tile_pool(name="kt_ps", bufs=4, space="PSUM") as kt_pool, \
             tc.tile_pool(name="sc_ps", bufs=2, space="PSUM") as sc_pool, \
             tc.tile_pool(name="pt_ps", bufs=1, space="PSUM") as pt_pool, \
             tc.tile_pool(name="cx_ps", bufs=1, space="PSUM") as cx_pool:

            # ---------- superblock loader (2 seq blocks per DMA) ----------
            supers = {}
            first_xps = []

            def ensure_super(b, sbk):
                key = (b, sbk)
                if key in supers:
                    return supers[key]
                s0 = sbk * 2 * SBLK
                v_nat = vn_pool.tile([128, 2 * NT, VD], F16, tag="vn",
                                     name=f"vn_{b}_{sbk}")
                # (p t) order: partition p holds 8 consecutive seq rows, so
                # each partition reads one 16KB-contiguous DRAM run.
                # v_nat[p, t, v] = value[b, s0 + 8p + t, v]
                src = value[b, s0:s0 + 2 * SBLK, :].rearrange(
                    "(p t) v -> p t v", p=128)
                if sbk == 0:
                    # latency-critical first superblock of a batch: split per
                    # half so the first transpose starts sooner
                    for h in range(2):
                        nc.gpsimd.dma_start(
                            out=v_nat[:, 4 * h:4 * h + 4, :],
                            in_=src[:, 4 * h:4 * h + 4, :])
                else:
                    ld = nc.gpsimd.dma_start(out=v_nat[:], in_=src)
                    if b == 0 and sbk in (1, 2) and first_xps:
                        # keep the kernel-start copy phase short: the bulk
                        # prefetch loads queue behind the first transposes
                        for xp in first_xps:
                            _dep(ld, xp)
                # vT8[p,tg,c,q] = value[b, s0 + 8q + tg, c*128+p], tg in [0,8)
                # (block j = 2*sbk+h owns tg = 4h+t; score column n = t*128+q)
                vT8 = vt_pool.tile([128, 2 * NT, NVC, 128], F16, tag="vt4",
                                   name=f"vT8_{b}_{sbk}")
                if sbk == 0:
                    # per-half transposes so the first k-matmuls start sooner
                    for h in range(2):
                        xp = nc.sync.dma_start(
                            out=vT8[:, h * NT:(h + 1) * NT, :, :].rearrange(
                                "p t c s -> p (t c) s"),
                            in_=v_nat[:, h * NT:(h + 1) * NT, :].rearrange(
                                "p t v -> p (t v)"),
                            transpose=True)
                        if b == 0:
                            first_xps.append(xp)
                else:
                    nc.sync.dma_start(
                        out=vT8[:].rearrange("p t c s -> p (t c) s"),
                        in_=v_nat[:].rearrange("p t v -> p (t v)"),
                        transpose=True)
                supers[key] = (v_nat, vT8)
                return supers[key]

            # ---------- persistent constants ----------
            wk16 = pp.tile([128, NVC, HD], F16)
            ones16 = pp.tile([1, BPC], F16)
            onesf = pp.tile([1, 1], F32)
            woT16 = pp.tile([128, NHC], F16)
            bo_sb = pp.tile([1, 1], F32)
            bo4 = pp.tile([1, 1], F32)
            qt_sb = pp.tile([128, NHC, BPC], F32)
            wq16 = pp.tile([128, NQC, HD], F16)
            qT16 = pp.tile([128, NQC, BPC], F16)
            bq16 = pp.tile([1, HD], F16)
            wo16 = pp.tile([1, HD], F16)

            # prefetch the first superblock before any constant loads
            ensure_super(0, 0)

            # ---------- preamble ----------
            nc.vector.memset(ones16[:], 1.0)
            nc.vector.memset(onesf[:], 1.0)
            nc.gpsimd.dma_start(out=wo16[:], in_=wo[:, :])
            nc.gpsimd.dma_start(out=bq16[:], in_=bq[:, :])
            nc.gpsimd.dma_start(out=bo_sb[:], in_=bo[:, :])
            nc.vector.tensor_scalar_add(bo4[:], bo_sb[:], EXP_SHIFT)
            for qc in range(NQC):
                nc.gpsimd.dma_start(
                    out=qT16[:, qc, :],
                    in_=query[:, qc * 128:(qc + 1) * 128].rearrange("b p -> p b"))
            nc.gpsimd.dma_start(out=wq16[:],
                                in_=wq[:, :].rearrange("(c p) h -> p c h", p=128))
            for vc in range(NVC):
                nc.gpsimd.dma_start(out=wk16[:, vc, :],
                                    in_=wk[vc * 128:(vc + 1) * 128, :])
            woT_ps = pt_pool.tile([128, NHC], F32, tag="ptps")
            for hc in range(NHC):
                nc.tensor.matmul(woT_ps[:, hc:hc + 1],
                                 wo16[:, hc * 128:(hc + 1) * 128], ones16[:, :1])
            nc.vector.tensor_copy(woT16[:], woT_ps[:])
            for hc in range(NHC):
                qt_ps = pt_pool.tile([128, BPC], F32, tag="ptps",
                                     name=f"qt_ps_{hc}")
                for qc in range(NQC):
                    nc.tensor.matmul(qt_ps[:],
                                     wq16[:, qc, hc * 128:(hc + 1) * 128],
                                     qT16[:, qc, :],
                                     start=(qc == 0), stop=False)
                nc.tensor.matmul(qt_ps[:], bq16[:, hc * 128:(hc + 1) * 128],
                                 ones16[:, :BPC], start=False, stop=True)
                nc.vector.tensor_copy(qt_sb[:, hc, :], qt_ps[:])

            # ---------- software-pipelined main loop ----------
            batch_state = {}

            def get_batch(b):
                if b not in batch_state:
                    mask_row = rows.tile([1, S], F16, tag="mask",
                                         name=f"mask_{b}")
                    nc.gpsimd.dma_start(out=mask_row[:], in_=mask[b:b + 1, :])
                    batch_state[b] = dict(
                        mask_row=mask_row,
                        p_row=rows.tile([1, S], F32, tag="p", name=f"p_{b}"),
                        l_parts=rows.tile([1, NBLK], F32, tag="lp",
                                          name=f"lp_{b}"),
                        ctx_acc=rows.tile([1, VD], F32, tag="ctxacc",
                                          name=f"cacc_{b}"),
                    )
                return batch_state[b]

            def emit_head(b, j):
                get_batch(b)
                v_nat, vT8 = ensure_super(b, j // 2)
                h = j % 2
                hT = ht_pool.tile([128, NHC, SBLK], F16, tag="ht",
                                  name=f"hT_{b}_{j}")
                for hc in range(NHC):
                    kt_ps = kt_pool.tile([128, SBLK], F32, tag="kt",
                                         name=f"kt_{b}_{j}_{hc}")
                    for vc in range(NVC):
                        nc.tensor.matmul(kt_ps[:],
                                         wk16[:, vc, hc * 128:(hc + 1) * 128],
                                         vT8[:, h * NT:(h + 1) * NT, vc, :],
                                         start=(vc == 0), stop=(vc == NVC - 1))
                    nc.scalar.activation(hT[:, hc, :], kt_ps[:], AF.Tanh,
                                         bias=qt_sb[:, hc, b:b + 1])
                return dict(b=b, j=j, h=h, hT=hT, v_nat=v_nat)

            def emit_tail(st):
                b, j, h, hT, v_nat = st["b"], st["j"], st["h"], st["hT"], st["v_nat"]
                bs = get_batch(b)
                sj = j * SBLK
                sc_ps = sc_pool.tile([1, SBLK], F32, tag="sc",
                                     name=f"sc_{b}_{j}")
                for hc in range(NHC):
                    nc.tensor.matmul(sc_ps[:], woT16[:, hc:hc + 1], hT[:, hc, :],
                                     start=(hc == 0), stop=(hc == NHC - 1))
                e_blk = rows.tile([1, SBLK], F32, tag="eblk", name=f"e_{b}_{j}")
                nc.scalar.activation(e_blk[:], sc_ps[:], AF.Exp, bias=bo4[:, :1])
                # mask in permuted order: column n = t*128+q <-> s = s0+8q+4h+t
                s0 = (j // 2) * 2 * SBLK
                mview = bs["mask_row"][0:1, s0:s0 + 2 * SBLK].rearrange(
                    "o (q x) -> o q x", q=128)[:, :, 4 * h:4 * h + 4].rearrange(
                    "o q t -> o t q")
                nc.vector.scalar_tensor_tensor(
                    bs["p_row"][:, sj:sj + SBLK], e_blk[:], 1.0,
                    mview,
                    op0=ALU.mult, op1=ALU.mult,
                    accum_out=bs["l_parts"][:, j:j + 1])
                pt_ps = pt_pool.tile([128, NT], F32, tag="ptps",
                                     name=f"pt_{b}_{j}")
                for t in range(NT):
                    nc.tensor.matmul(pt_ps[:, t:t + 1],
                                     bs["p_row"][:, sj + t * 128:sj + (t + 1) * 128],
                                     onesf[:, :1])
                pT16 = rows.tile([128, NT], F16, tag="pt16", name=f"pt16_{b}_{j}")
                nc.vector.tensor_copy(pT16[:], pt_ps[:])
                ctx_ps = cx_pool.tile([1, VD], F32, tag="cx", name=f"cx_{b}_{j}")
                for t in range(NT):
                    nc.tensor.matmul(ctx_ps[:], pT16[:, t:t + 1],
                                     v_nat[:, h * NT + t, :],
                                     start=(t == 0), stop=(t == NT - 1))
                if j == 0:
                    nc.vector.tensor_copy(bs["ctx_acc"][:], ctx_ps[:])
                else:
                    nc.vector.tensor_tensor(bs["ctx_acc"][:], bs["ctx_acc"][:],
                                            ctx_ps[:], op=ALU.add)
            def emit_epilogue(b):
                bs = batch_state[b]
                l_sb = rows.tile([1, 1], F32, tag="l", name=f"l_{b}")
                nc.vector.reduce_sum(l_sb[:], bs["l_parts"][:],
                                     axis=mybir.AxisListType.X)
                rinv = rows.tile([1, 1], F32, tag="rinv", name=f"rinv_{b}")
                nc.vector.reciprocal(rinv[:], l_sb[:])
                # unpermute p_row (sb hh t q order) to natural s order
                att_row = rows.tile([1, S], F32, tag="att", name=f"att_{b}")
                pview = bs["p_row"][0:1, :].rearrange(
                    "o (sb hh t q) -> o sb q hh t",
                    sb=NBLK // 2, hh=2, t=NT, q=128)
                nc.vector.tensor_scalar_mul(att_row[:], pview, rinv[:, :1])
                nc.gpsimd.dma_start(out=att_o[b:b + 1, :], in_=att_row[:])
                ctx_sb = rows.tile([1, VD], F32, tag="ctx", name=f"ctxsb_{b}")
                nc.vector.tensor_scalar_mul(ctx_sb[:], bs["ctx_acc"][:],
                                            rinv[:, :1])
                nc.gpsimd.dma_start(out=ctx_o[b:b + 1, :], in_=ctx_sb[:])

            blocks = [(b, j) for b in range(BPC) for j in range(NBLK)]
            pending = []
            epi_queue = []  # (batch, tails_remaining_until_emit)
            done_tails = 0

            def tail_done(st):
                nonlocal done_tails
                emit_tail(st)
                done_tails += 1
                if st["j"] == NBLK - 1:
                    epi_queue.append((st["b"], done_tails + EPI_LAG))
                while epi_queue and epi_queue[0][1] <= done_tails:
                    emit_epilogue(epi_queue.pop(0)[0])

            for (b, j) in blocks:
                pending.append(emit_head(b, j))
                if len(pending) > TAIL_LAG:
                    tail_done(pending.pop(0))
            for st in pending:
                tail_done(st)
            while epi_queue:
                emit_epilogue(epi_queue.pop(0)[0])

    nc.compile()
    return nc


def _get_nc():
    if "nc" not in _CACHE:
        _CACHE["nc"] = _build()
    return _CACHE["nc"]


def run(inputs, trace=False, tmpdir=None):
    nc = _get_nc()
    value = np.ascontiguousarray(np.asarray(inputs["value"], dtype=np.float32))
    query = np.ascontiguousarray(np.asarray(inputs["query"], dtype=np.float32))
    mask = np.ascontiguousarray(np.asarray(inputs["mask"]).astype(np.uint8))
    wk = np.ascontiguousarray(np.asarray(inputs["Wk"], dtype=np.float32))
    wq = np.ascontiguousarray(np.asarray(inputs["Wq"], dtype=np.float32))
    bq = np.asarray(inputs["bq"], dtype=np.float32).reshape(1, HD)
    wo = np.asarray(inputs["Wo"], dtype=np.float32).reshape(1, HD)
    bo = np.asarray(inputs["bo"], dtype=np.float32).reshape(1, 1)

    in_maps = []
    for c in range(NCORES):
        sl = slice(c * BPC, (c + 1) * BPC)
        in_maps.append({
            "value_s": value[sl],
            "query_s": query[sl],
            "mask_s": mask[sl],
            "Wk": wk, "Wq": wq, "bq": bq, "Wo": wo, "bo": bo,
        })
    res = run_bass_kernel_spmd(nc, in_maps, core_ids=list(range(NCORES)),
                               trace=trace, tmpdir=tmpdir)
    ctx = np.concatenate([r["ctx_s"] for r in res.results], axis=0)
    att = np.concatenate([r["att_s"] for r in res.results], axis=0)
    return (ctx, att), res


def kernel(**inputs):
    (ctx, att), _ = run(inputs)
    return ctx, att


if __name__ == "__main__":
    rng = np.random.default_rng(0)
    demo = {
        "query": rng.standard_normal((B, QD)).astype(np.float32),
        "value": rng.standard_normal((B, S, VD)).astype(np.float32),
        "mask": np.ones((B, S), dtype=bool),
        "Wk": rng.uniform(-1, 1, (VD, HD)).astype(np.float32) / np.sqrt(VD),
        "Wq": rng.uniform(-1, 1, (QD, HD)).astype(np.float32) / np.sqrt(QD),
        "bq": rng.uniform(-1, 1, HD).astype(np.float32) / np.sqrt(QD),
        "Wo": rng.uniform(-1, 1, HD).astype(np.float32) / np.sqrt(HD),
        "bo": np.float32(0.01),
    }
    ctx, att = kernel(**demo)
    print("ctx", ctx.shape, "att", att.shape)
